# revision 10
# baseline (speedup 1.0000x reference)
"""Trainium2 Bass kernel for the MERITS_T patient model (B=1024 data-parallel over 8 cores).

Mathematical simplification of the reference (verified to ~7e-7 rel err in f32,
~4.3e-3 in bf16 against the jax reference; tolerance is 2e-2):
  - E_de = _mha(drug_mem, e0, e0) softmaxes over a single key, so its output is
    e0 @ m2_wv @ m2_wo broadcast over all 145 query rows -> the three GATs, the
    graph MHA and drug_mem never reach the output (dead code).
  - e0 = E_en[:, 0] only needs query row 0 of the m1 attention, i.e. only the
    first visit of `med`.
  - patient_j = [glu_rep_j ; static]: the static half is visit-independent, so
    it cancels in the softmax over visits and contributes static @ SMW to the
    output (SMW = sum_h (wv_h wo_h m2_wv m2_wo)[32:, :]); only the 32 glu dims
    participate in scores and the attention-weighted sum.
  - final reshape tiles r 145x, so relu(final) @ out_w1 = relu(r) @ sum_m
    out_w1[m]. The 43MB sum over m is column-sharded 8 ways: each core reduces
    its own 145-column slice fully on-device and a small bf16 AllGather
    (18.5KB/core) assembles the full [64, 1160] on every core.

All arithmetic runs on device (bf16 compute, f32 PSUM accumulation); the host
only marshals layouts (transpose / pad / concat / replicate / constant fill).
"""

import numpy as np
import ml_dtypes

import concourse.bass as bass
import concourse.mybir as mybir
from concourse.bass_utils import run_bass_kernel_spmd
from concourse.tile import TileContext

F32 = mybir.dt.float32
BF16 = mybir.dt.bfloat16
AF = mybir.ActivationFunctionType
ALU = mybir.AluOpType
AX = mybir.AxisListType


def split_multi_waits(nc):
    """The walrus on this image encodes at most ONE sync wait per TPB
    instruction ("Too many sync wait commands" otherwise). Hoist excess waits
    onto standalone InstEventSemaphore ops on the same engine, immediately
    before the instruction - the same mechanism Tile's barriers use."""
    wid = 0
    for f in nc.m.functions:
        for bb in f.blocks:
            out = []
            for ins in bb.instructions:
                si = ins.sync_info
                if si is not None and si.on_wait and len(si.on_wait) > 1:
                    waits = list(si.on_wait)
                    for w in waits[:-1]:
                        wid += 1
                        out.append(mybir.InstEventSemaphore(
                            name=f"Wsplit-{wid}", engine=ins.engine,
                            ins=[], outs=[],
                            sync_info=mybir.SyncInfo(on_wait=[w], on_update=[])))
                    si.on_wait = waits[-1:]
                out.append(ins)
            bb.instructions = out
    return wid


B, T, MED, LAB, GLU, D, H = 1024, 25, 145, 1956, 16, 64, 32
NC_CORES = 8
BC = B // NC_CORES          # 128 patients per core
NH, DH = 4, 16
HID = MED * D // 8          # 1160
CW = HID // NC_CORES        # 145 W1s columns per core

# blob column map (f32 [128, BK]; one on-device bf16 cast, then sliced)
C_WBDG, C_WBDT = 0, 256          # block-diag glu/tf weights    [r0:128]
C_GG, C_GB, C_MG = 512, 544, 576  # glu_gate/glu_b/med_gate rep [r0:128]
C_MWA, C_B1T = 640, 704          # med_w rows 0:128; b1T        [r0:128]
C_MWB = 714                      # med_w rows 128:146           [r0:18]
C_B2 = 778                       # out_b2 row                   [r0:1]
C_WOT, C_M2WVT, C_M2WO = 923, 987, 1051   # [r0:64]
C_SLW2 = 1115                    # sll_w2 + bias row            [r0:65]
C_WQT, C_WKT, C_WVT = 1147, 1403, 1531    # per-head q/k/v^T     [r0:16]
C_GW3G, C_GW3T = 1787, 1819      # visit-24 glu/tf weights      [r0:16]
BK = 1851


def build_bass(split_waits=True):
    nc = bass.Bass()

    def inp(name, shape):
        return nc.dram_tensor(name, list(shape), F32, kind="ExternalInput")

    # ---- per-core inputs (host-marshalled layouts) ----
    w1m_d = inp("w1m", (128, 19, 580))     # out_w1 m-shard, (half,d)-major
    labt_d = inp("labt", (BC, 16, 128))    # lab^T k-tiles (+ones col folded)
    slwt_d = inp("slwt", (BC, 16, D))      # sll_w1 k-tiles (+bias row folded)
    glut_d = inp("glut", (BC, 4, BC))      # glu (j,f)-major transpose
    tft_d = inp("tft", (BC, 4, BC))        # time_feat ditto
    medt_d = inp("medt", (MED + 1, BC))    # med visit-0 transposed + ones row
    w2t_d = inp("w2t", (BC, 10, MED))      # out_w2 k-tiles (1160 rows 0-padded)
    blob_d = inp("blob", (BC, BK))         # packed small weights

    identity = nc.inline_tensor(np.eye(128, dtype=ml_dtypes.bfloat16),
                                name="ident128")

    # collective buffers (DRAM); AllReduce sums the 8 partial W1s
    cc_in = nc.dram_tensor("cc_in", [D, HID], F32)
    cc_out = nc.dram_tensor("cc_out", [D, HID], F32, addr_space="Shared")
    out_d = nc.dram_tensor("out", [BC, MED], F32, kind="ExternalOutput")

    with TileContext(nc) as tc, \
            tc.tile_pool(name="consts", bufs=1) as cp, \
            tc.tile_pool(name="ps", bufs=3, space="PSUM") as ps, \
            tc.tile_pool(name="psg", bufs=1, space="PSUM") as psg, \
            tc.tile_pool(name="pst1", bufs=1, space="PSUM") as pst1, \
            tc.tile_pool(name="pout", bufs=1, space="PSUM") as pout:

        # ================= DMA issue (order = queue execution order) ========
        # Pool/gpsimd queue: the big w1 m-shard, cast f32->bf16 in-flight
        w1m_b = cp.tile([128, 19, 580], BF16, tag="w1m_b")
        nc.gpsimd.dma_start(out=w1m_b[:, 0:10, :], in_=w1m_d[:, 0:10, :])
        nc.gpsimd.dma_start(out=w1m_b[:, 10:19, :], in_=w1m_d[:, 10:19, :])

        # SP queue: blob first (unblocks all weight prep), then big f32 tiles
        blob_f = cp.tile([BC, BK], F32, tag="blob_f")
        nc.sync.dma_start(out=blob_f, in_=blob_d[:])
        labt_f = cp.tile([BC, 16, 128], F32, tag="labt_f")
        nc.sync.dma_start(out=labt_f, in_=labt_d[:])
        slwt_f = cp.tile([BC, 16, D], F32, tag="slwt_f")
        nc.sync.dma_start(out=slwt_f, in_=slwt_d[:])
        w2t_f = cp.tile([BC, 10, MED], F32, tag="w2t_f")
        nc.sync.dma_start(out=w2t_f, in_=w2t_d[:])

        # Act queue: small fast tensors feeding the longest compute chain
        glut_f = cp.tile([BC, 4, BC], F32, tag="glut_f")
        nc.scalar.dma_start(out=glut_f, in_=glut_d[:])
        tft_f = cp.tile([BC, 4, BC], F32, tag="tft_f")
        nc.scalar.dma_start(out=tft_f, in_=tft_d[:])
        med_fa = cp.tile([BC, BC], F32, tag="med_fa")
        nc.scalar.dma_start(out=med_fa, in_=medt_d[0:128, :])
        med_fb = cp.tile([18, BC], F32, tag="med_fb")
        nc.scalar.dma_start(out=med_fb, in_=medt_d[128:146, :])
        ident = cp.tile([128, 128], BF16, tag="ident")
        nc.scalar.dma_start(out=ident, in_=identity[:])

        # ================= bf16 casts (Act engine, readiness order) =========
        glut_b = cp.tile([BC, 4, BC], BF16, tag="glut_b")
        nc.scalar.copy(out=glut_b, in_=glut_f)
        tft_b = cp.tile([BC, 4, BC], BF16, tag="tft_b")
        nc.scalar.copy(out=tft_b, in_=tft_f)
        blob_b = cp.tile([BC, BK], BF16, tag="blob_b")
        nc.scalar.copy(out=blob_b, in_=blob_f)
        labt_b = cp.tile([BC, 16, 128], BF16, tag="labt_b")
        nc.scalar.copy(out=labt_b, in_=labt_f)
        slwt_b = cp.tile([BC, 16, D], BF16, tag="slwt_b")
        nc.scalar.copy(out=slwt_b, in_=slwt_f)

        # ================= med multi-hot (DVE) ==============================
        mb_a = cp.tile([BC, BC], BF16, tag="mb_a")
        nc.vector.tensor_scalar(out=mb_a, in0=med_fa, scalar1=0.9,
                                scalar2=None, op0=ALU.is_gt)
        mb_b = cp.tile([18, BC], BF16, tag="mb_b")
        nc.vector.tensor_scalar(out=mb_b, in0=med_fb, scalar1=0.9,
                                scalar2=None, op0=ALU.is_gt)

        # ================= glu encoder matmuls (PE block-diagonal) ==========
        gx_ps = psg.tile([BC, T, H], F32, tag="gx")
        for c in range(3):
            nc.tensor.matmul(gx_ps[:, 8 * c:8 * c + 8, :],
                             lhsT=glut_b[:, c, :],
                             rhs=blob_b[:, C_WBDG:C_WBDG + 256],
                             start=True, stop=False)
            nc.tensor.matmul(gx_ps[:, 8 * c:8 * c + 8, :],
                             lhsT=tft_b[:, c, :],
                             rhs=blob_b[:, C_WBDT:C_WBDT + 256],
                             start=False, stop=True)
        nc.tensor.matmul(gx_ps[:, 24, :], lhsT=glut_b[0:16, 3, :],
                         rhs=blob_b[0:16, C_GW3G:C_GW3G + 32],
                         start=True, stop=False)
        nc.tensor.matmul(gx_ps[:, 24, :], lhsT=tft_b[0:16, 3, :],
                         rhs=blob_b[0:16, C_GW3T:C_GW3T + 32],
                         start=False, stop=True)

        # med x0 = multihot @ med_w + med_b (bias row folded via ones row)
        x0_ps = ps.tile([BC, D], F32, tag="acc")
        nc.tensor.matmul(x0_ps, lhsT=mb_a,
                         rhs=blob_b[:, C_MWA:C_MWA + D], start=True, stop=False)
        nc.tensor.matmul(x0_ps, lhsT=mb_b,
                         rhs=blob_b[0:18, C_MWB:C_MWB + D], start=False, stop=True)

        # ================= weight prep (PE + gpsimd copies) =================
        # A_h[:, :32] = wq_h wk_h^T / 4 (glu columns only), stacked [64,(h,32)]
        a_ps = ps.tile([D, NH, H], F32, tag="acc")
        for h in range(NH):
            nc.tensor.matmul(a_ps[:, h, :],
                             lhsT=blob_b[0:16, C_WQT + 64 * h:C_WQT + 64 * h + 64],
                             rhs=blob_b[0:16, C_WKT + 32 * h:C_WKT + 32 * h + 32])
        # Wvo2 = m2_wv @ m2_wo
        wvo_ps = ps.tile([D, D], F32, tag="acc")
        nc.tensor.matmul(wvo_ps, lhsT=blob_b[0:D, C_M2WVT:C_M2WVT + D],
                         rhs=blob_b[0:D, C_M2WO:C_M2WO + D])

        a_sb = cp.tile([D, NH, H], BF16, tag="a_sb")
        nc.scalar.activation(out=a_sb, in_=a_ps, func=AF.Copy, scale=0.25)
        wvo_sb = cp.tile([D, D], BF16, tag="wvo_sb")
        nc.scalar.copy(out=wvo_sb, in_=wvo_ps)

        # t_h = wo_h @ Wvo2  [16,(h),64]
        t_ps = ps.tile([16, NH, D], F32, tag="acc")
        for h in range(NH):
            nc.tensor.matmul(t_ps[:, h, :],
                             lhsT=blob_b[0:D, C_WOT + 16 * h:C_WOT + 16 * h + 16],
                             rhs=wvo_sb)
        t_sb = cp.tile([16, NH, D], BF16, tag="t_sb")
        nc.scalar.copy(out=t_sb, in_=t_ps)
        # mwT[e, h, f] = MW_h[f, e] = (wv_h t_h)[f, e], computed transposed
        mwt_ps = ps.tile([D, NH, D], F32, tag="acc")
        for h in range(NH):
            nc.tensor.matmul(mwt_ps[:, h, :], lhsT=t_sb[:, h, :],
                             rhs=blob_b[0:16, C_WVT + 64 * h:C_WVT + 64 * h + 64])
        mwtg_sb = cp.tile([D, NH, H], BF16, tag="mwtg_sb")
        nc.scalar.copy(out=mwtg_sb, in_=mwt_ps[:, :, 0:H])
        mwth_sb = cp.tile([D, NH, H], BF16, tag="mwth_sb")
        nc.scalar.copy(out=mwth_sb, in_=mwt_ps[:, :, H:D])
        # MWg_stack[(h,f<32), e] via one PE transpose of mwT[:, :, :32]
        mwg_ps = ps.tile([128, D], BF16, tag="acc")
        nc.tensor.transpose(mwg_ps, mwtg_sb[:].rearrange("p h f -> p (h f)"),
                            ident[0:D, 0:D])
        mwg_sb = cp.tile([128, D], BF16, tag="mwg_sb")
        nc.scalar.copy(out=mwg_sb, in_=mwg_ps)
        # SMW = sum_h MW_h[32:, :]: sum mwT halves then transpose
        smwt = cp.tile([D, H], BF16, tag="smwt")
        nc.gpsimd.tensor_tensor(out=smwt, in0=mwth_sb[:, 0, :],
                                in1=mwth_sb[:, 1, :], op=ALU.add)
        nc.gpsimd.tensor_tensor(out=smwt, in0=smwt, in1=mwth_sb[:, 2, :],
                                op=ALU.add)
        nc.gpsimd.tensor_tensor(out=smwt, in0=smwt, in1=mwth_sb[:, 3, :],
                                op=ALU.add)
        smw_ps = ps.tile([H, D], BF16, tag="acc")
        nc.tensor.transpose(smw_ps, smwt[:], ident[0:D, 0:D])
        smw_sb = cp.tile([H, D], BF16, tag="smw_sb")
        nc.scalar.copy(out=smw_sb, in_=smw_ps)
        st1rt = cp.tile([D + 1, BC], BF16, tag="st1rt")
        nc.gpsimd.memset(st1rt[D:D + 1, :], 1.0)

        # ================= med gating -> mr0 -> u (critical chain) ==========
        scr = cp.tile([BC, D], BF16, tag="scr")
        nc.vector.tensor_mul(scr, x0_ps, blob_b[:, C_MG:C_MG + D])
        g0 = cp.tile([BC, 1], F32, tag="g0")
        nc.vector.tensor_reduce(out=g0, in_=scr, axis=AX.X, op=ALU.add)
        sg0 = cp.tile([BC, 1], F32, tag="sg0")
        nc.scalar.activation(out=sg0, in_=g0, func=AF.Sigmoid)
        mr0 = cp.tile([BC, D], BF16, tag="mr0")
        nc.vector.tensor_scalar(out=mr0, in0=x0_ps, scalar1=sg0[:, 0:1],
                                scalar2=None, op0=ALU.mult)
        mr0t_ps = ps.tile([D, BC], BF16, tag="acc")
        nc.tensor.transpose(mr0t_ps, mr0[:], ident[:])
        mr0t = cp.tile([D, BC], BF16, tag="mr0t")
        nc.vector.tensor_copy(out=mr0t, in_=mr0t_ps)
        u_ps = ps.tile([BC, NH, H], F32, tag="acc")
        nc.tensor.matmul(u_ps[:].rearrange("p h e -> p (h e)"), lhsT=mr0t,
                         rhs=a_sb[:].rearrange("p h e -> p (h e)"))
        u_sb = cp.tile([BC, NH, H], BF16, tag="u_sb")
        nc.vector.tensor_copy(out=u_sb, in_=u_ps)

        # ================= glu encoder tail (DVE/Act) =======================
        gxb = cp.tile([BC, T, H], BF16, tag="gxb")
        nc.vector.tensor_add(gxb, gx_ps,
                             blob_b[:, C_GB:C_GB + H].unsqueeze(1)
                             .broadcast_to((BC, T, H)))
        grep = cp.tile([BC, T, H], BF16, tag="grep")
        nc.scalar.activation(out=grep, in_=gxb, func=AF.Tanh)
        gm = cp.tile([BC, T, H], BF16, tag="gm")
        nc.vector.tensor_mul(gm, grep,
                             blob_b[:, C_GG:C_GG + H].unsqueeze(1)
                             .broadcast_to((BC, T, H)))
        gs = cp.tile([BC, T], F32, tag="gs")
        nc.vector.tensor_reduce(out=gs, in_=gm, axis=AX.X, op=ALU.add)
        gsg = cp.tile([BC, T], BF16, tag="gsg")
        nc.scalar.activation(out=gsg, in_=gs, func=AF.Sigmoid)
        nc.vector.tensor_mul(grep, grep,
                             gsg[:].unsqueeze(2).broadcast_to((BC, T, H)))

        # ================= one-query attention (glu dims only) ==============
        sprod = cp.tile([BC, NH, T, H], BF16, tag="sprod")
        nc.vector.tensor_mul(
            sprod,
            grep[:].unsqueeze(1).broadcast_to((BC, NH, T, H)),
            u_sb[:].unsqueeze(2).broadcast_to((BC, NH, T, H)))
        # halving-tree reduce over f (keeps fast bf16 DVE throughput)
        for wdt in (16, 8, 4, 2, 1):
            nc.vector.tensor_add(sprod[:, :, :, 0:wdt], sprod[:, :, :, 0:wdt],
                                 sprod[:, :, :, wdt:2 * wdt])
        es = cp.tile([BC, NH, T], BF16, tag="es")
        nc.scalar.activation(out=es, in_=sprod[:, :, :, 0], func=AF.Exp)
        den = cp.tile([BC, NH], F32, tag="den")
        nc.vector.tensor_reduce(out=den, in_=es, axis=AX.X, op=ALU.add)
        rden = cp.tile([BC, NH], F32, tag="rden")
        nc.vector.reciprocal(out=rden, in_=den)
        attn = cp.tile([BC, NH, T], BF16, tag="attn")
        nc.vector.tensor_mul(attn, es,
                             rden[:].unsqueeze(2).broadcast_to((BC, NH, T)))
        # weighted sum over visits, f-major so innermost stays packed
        grept = cp.tile([BC, H, T], BF16, tag="grept")
        nc.vector.tensor_copy(out=grept,
                              in_=grep[:].rearrange("p j f -> p f j"))
        wprod = cp.tile([BC, NH, H, T], BF16, tag="wprod")
        nc.vector.tensor_mul(
            wprod,
            attn[:].unsqueeze(2).broadcast_to((BC, NH, H, T)),
            grept[:].unsqueeze(1).broadcast_to((BC, NH, H, T)))
        nc.vector.tensor_add(wprod[:, :, :, 0:9], wprod[:, :, :, 0:9],
                             wprod[:, :, :, 16:25])
        for wdt in (8, 4, 2, 1):
            nc.vector.tensor_add(wprod[:, :, :, 0:wdt], wprod[:, :, :, 0:wdt],
                                 wprod[:, :, :, wdt:2 * wdt])
        wfin = cp.tile([BC, NH, H], BF16, tag="wfin")
        nc.vector.tensor_copy(out=wfin[:].unsqueeze(3), in_=wprod[:, :, :, 0:1])

        # ================= W1s partial reduce + AllReduce ===================
        def tree(lo, n):  # reduce w1m_b[:, lo:lo+n] into w1m_b[:, lo]
            while n > 1:
                half, odd = n // 2, n % 2
                if odd:
                    nc.vector.tensor_add(w1m_b[:, lo:lo + 1, :],
                                         w1m_b[:, lo:lo + 1, :],
                                         w1m_b[:, lo + n - 1:lo + n, :])
                    n -= 1
                nc.vector.tensor_add(w1m_b[:, lo:lo + half, :],
                                     w1m_b[:, lo:lo + half, :],
                                     w1m_b[:, lo + half:lo + 2 * half, :])
                n = half

        tree(0, 10)
        tree(10, 9)
        w1p = cp.tile([128, 580], BF16, tag="w1p")
        nc.vector.tensor_add(w1p[:].unsqueeze(1), w1m_b[:, 0:1, :],
                             w1m_b[:, 10:11, :])
        w1pf = cp.tile([128, 580], F32, tag="w1pf")
        nc.scalar.copy(out=w1pf, in_=w1p)
        nc.scalar.dma_start(
            out=cc_in[:].rearrange("d (r q) -> r d q", r=2), in_=w1pf)
        nc.gpsimd.collective_compute(
            "AllReduce", ALU.add, replica_groups=[list(range(NC_CORES))],
            ins=[cc_in[:]], outs=[cc_out[:]])

        # ================= static MLP (transposed; no lab transpose) ========
        st1_ps = pst1.tile([D, BC], F32, tag="st1")
        for t in range(16):
            nc.tensor.matmul(st1_ps, lhsT=slwt_b[:, t, :], rhs=labt_b[:, t, :],
                             start=(t == 0), stop=(t == 15))
        nc.scalar.activation(out=st1rt[0:D, :], in_=st1_ps, func=AF.Relu)
        stat_ps = ps.tile([H, BC], F32, tag="acc")
        nc.tensor.matmul(stat_ps, lhsT=blob_b[0:D + 1, C_SLW2:C_SLW2 + H],
                         rhs=st1rt)
        statt = cp.tile([H, BC], BF16, tag="statt")
        nc.scalar.activation(out=statt, in_=stat_ps, func=AF.Relu)

        # ================= r = attention out + static part ==================
        wgt_ps = ps.tile([128, BC], BF16, tag="acc")
        nc.tensor.transpose(wgt_ps, wfin[:].rearrange("p h f -> p (h f)"),
                            ident[:])
        wgt_sb = cp.tile([128, BC], BF16, tag="wgt_sb")
        nc.vector.tensor_copy(out=wgt_sb, in_=wgt_ps)
        r_ps = ps.tile([BC, D], F32, tag="acc")
        nc.tensor.matmul(r_ps, lhsT=statt, rhs=smw_sb, start=True, stop=False)
        nc.tensor.matmul(r_ps, lhsT=wgt_sb, rhs=mwg_sb, start=False, stop=True)
        rr = cp.tile([BC, D], BF16, tag="rr")
        nc.scalar.activation(out=rr, in_=r_ps, func=AF.Relu)
        rrt_ps = ps.tile([D, BC], BF16, tag="acc")
        nc.tensor.transpose(rrt_ps, rr[:], ident[:])
        rrt = cp.tile([D, BC], BF16, tag="rrt")
        nc.vector.tensor_copy(out=rrt, in_=rrt_ps)

        # w2 cast late on Act queue (only needed after the AllGather)
        w2t_b = cp.tile([BC, 10, MED], BF16, tag="w2t_b")
        nc.scalar.copy(out=w2t_b, in_=w2t_f)
        ones_sb = cp.tile([1, BC], BF16, tag="ones_sb")
        nc.gpsimd.memset(ones_sb, 1.0)

        # ================= final MLP (after AllGather) ======================
        w1s_sb = cp.tile([D, HID], BF16, tag="w1s_sb")
        nc.gpsimd.dma_start(out=w1s_sb, in_=cc_out[:])
        hidt = cp.tile([128, 10, 128], BF16, tag="hidt")
        # phase 1: all hidT matmuls + relus (pipelined, no out-matmul in between)
        for t in range(10):
            w = 128 if t < 9 else 8
            h_ps = ps.tile([128, BC], F32, tag="acc")
            nc.tensor.matmul(h_ps[0:w, :],
                             lhsT=w1s_sb[:, 128 * t:128 * t + w], rhs=rrt)
            nc.scalar.activation(out=hidt[0:w, t, :], in_=h_ps[0:w, :],
                                 func=AF.Relu,
                                 bias=blob_b[0:w, C_B1T + t:C_B1T + t + 1])
        # phase 2: back-to-back output accumulation
        out_ps = pout.tile([BC, MED], F32, tag="outacc")
        nc.tensor.matmul(out_ps, lhsT=ones_sb,
                         rhs=blob_b[0:1, C_B2:C_B2 + MED],
                         start=True, stop=False, skip_group_check=True)
        for t in range(10):
            w = 128 if t < 9 else 8
            nc.tensor.matmul(out_ps, lhsT=hidt[0:w, t, :],
                             rhs=w2t_b[0:w, t, :],
                             start=False, stop=(t == 9), skip_group_check=True)
        out_sb = cp.tile([BC, MED], F32, tag="out_sb")
        nc.scalar.copy(out=out_sb, in_=out_ps)
        nc.scalar.dma_start(out=out_d[:], in_=out_sb)

    if split_waits:
        split_multi_waits(nc)
    return nc


_CACHED_NC = None


def make_in_maps(inputs):
    """Host-side input marshalling: pure layout work (transpose / pad / concat
    / replicate / constant fill) - every arithmetic op stays on device."""
    f = lambda x: np.ascontiguousarray(np.asarray(x, dtype=np.float32))
    cat = np.concatenate

    # out_w1 [9280, 1160] -> [145, 64, 1160] -> per-core m-shard of 19 blocks,
    # laid out [(half, d), m, 580] so the on-device tree reduce is contiguous
    w1 = f(inputs["out_w1"]).reshape(MED, D, HID)
    w1pad = np.zeros((8 * 19, D, HID), np.float32)
    w1pad[:MED] = w1

    # lab^T k-tiles with ones column folded at row 1957
    lab = f(inputs["lab"])
    lab_ext = np.zeros((B, 2048), np.float32)
    lab_ext[:, :LAB] = lab
    lab_ext[:, LAB] = 1.0           # ones column folds sll_b1 into the matmul

    slw = np.zeros((2048, D), np.float32)
    slw[:LAB] = f(inputs["sll_w1"])
    slw[LAB] = f(inputs["sll_b1"])
    slwt = np.ascontiguousarray(slw.reshape(16, 128, D).transpose(1, 0, 2))

    glu, tf = f(inputs["glu"]), f(inputs["time_feat"])

    def jf_major(x):  # (j, f)-major transpose, padded 400 -> 512
        z = np.zeros((B, 512), np.float32)
        z[:, :T * GLU] = x.reshape(B, T * GLU)
        return z.reshape(B, 4, 128).transpose(2, 1, 0)  # [128p, 4c, B]

    glut, tft = jf_major(glu), jf_major(tf)

    med0 = f(inputs["med"])[:, 0, :]                  # [B, 145]
    medt = np.ones((MED + 1, B), np.float32)          # row 145 = 1.0
    medt[:MED] = med0.T

    w2 = np.zeros((1280, MED), np.float32)
    w2[:HID] = f(inputs["out_w2"])
    w2t = np.ascontiguousarray(w2.reshape(10, 128, MED).transpose(1, 0, 2))

    # ---- packed weight blob ----
    blob = np.zeros((BC, BK), np.float32)
    glu_w = f(inputs["glu_w"])                         # [32, 32]
    for jl in range(8):
        blob[16 * jl:16 * jl + 16,
             C_WBDG + 32 * jl:C_WBDG + 32 * jl + 32] = glu_w[:16]
        blob[16 * jl:16 * jl + 16,
             C_WBDT + 32 * jl:C_WBDT + 32 * jl + 32] = glu_w[16:]
    blob[:, C_GG:C_GG + H] = f(inputs["glu_gate"])[None, :]
    blob[:, C_GB:C_GB + H] = f(inputs["glu_b"])[None, :]
    blob[:, C_MG:C_MG + D] = f(inputs["med_gate"])[None, :]
    medw_ext = cat([f(inputs["med_w"]), f(inputs["med_b"])[None, :]], 0)
    blob[:, C_MWA:C_MWA + D] = medw_ext[:128]
    blob[0:18, C_MWB:C_MWB + D] = medw_ext[128:]
    b1 = f(inputs["out_b1"])
    for t in range(10):
        nvalid = 128 if t < 9 else 8
        blob[:nvalid, C_B1T + t] = b1[128 * t:128 * t + nvalid]
    blob[0, C_B2:C_B2 + MED] = f(inputs["out_b2"])
    wo, wv = f(inputs["m1_wo"]), f(inputs["m1_wv"])
    wq, wk = f(inputs["m1_wq"]), f(inputs["m1_wk"])
    blob[0:D, C_WOT:C_WOT + D] = wo.T                  # woT[d, (h,g)]
    blob[0:D, C_M2WVT:C_M2WVT + D] = f(inputs["m2_wv"]).T
    blob[0:D, C_M2WO:C_M2WO + D] = f(inputs["m2_wo"])
    blob[0:D, C_SLW2:C_SLW2 + H] = f(inputs["sll_w2"])
    blob[D, C_SLW2:C_SLW2 + H] = f(inputs["sll_b2"])
    for h in range(NH):
        blob[0:16, C_WQT + 64 * h:C_WQT + 64 * h + 64] = wq[:, 16 * h:16 * h + 16].T
        blob[0:16, C_WKT + 32 * h:C_WKT + 32 * h + 32] = wk[:H, 16 * h:16 * h + 16].T
        blob[0:16, C_WVT + 64 * h:C_WVT + 64 * h + 64] = wv[:, 16 * h:16 * h + 16].T
    blob[0:16, C_GW3G:C_GW3G + H] = glu_w[:16]
    blob[0:16, C_GW3T:C_GW3T + H] = glu_w[16:]

    in_maps = []
    for c in range(NC_CORES):
        sl = slice(c * BC, (c + 1) * BC)
        in_maps.append({
            "w1m": np.ascontiguousarray(
                w1pad[19 * c:19 * c + 19].reshape(19, D, 2, 580)
                .transpose(2, 1, 0, 3).reshape(128, 19, 580)),
            "labt": np.ascontiguousarray(
                lab_ext[sl].T.reshape(16, 128, BC).transpose(1, 0, 2)),
            "slwt": slwt,
            "glut": np.ascontiguousarray(glut[:, :, sl]),
            "tft": np.ascontiguousarray(tft[:, :, sl]),
            "medt": np.ascontiguousarray(medt[:, sl]),
            "w2t": w2t,
            "blob": blob,
        })
    return in_maps


def kernel(**inputs):
    global _CACHED_NC
    if _CACHED_NC is None:
        _CACHED_NC = build_bass()
    nc = _CACHED_NC
    in_maps = make_in_maps(inputs)
    res = run_bass_kernel_spmd(nc, in_maps, core_ids=list(range(NC_CORES)))
    return np.concatenate([res.results[c]["out"] for c in range(NC_CORES)], axis=0)


if __name__ == "__main__":
    import reference
    inp = reference.setup_inputs()
    out = kernel(**{k: np.asarray(v) for k, v in inp.items()})
    print("kernel output", out.shape, out.dtype)


# revision 11
# speedup vs baseline: 1.0988x; 1.0988x over previous
"""Trainium2 Bass kernel for the MERITS_T patient model (B=1024 data-parallel over 8 cores).

Mathematical simplification of the reference (verified to ~7e-7 rel err in f32,
~4.3e-3 in bf16 against the jax reference; tolerance is 2e-2):
  - E_de = _mha(drug_mem, e0, e0) softmaxes over a single key, so its output is
    e0 @ m2_wv @ m2_wo broadcast over all 145 query rows -> the three GATs, the
    graph MHA and drug_mem never reach the output (dead code).
  - e0 = E_en[:, 0] only needs query row 0 of the m1 attention, i.e. only the
    first visit of `med`.
  - patient_j = [glu_rep_j ; static]: the static half is visit-independent, so
    it cancels in the softmax over visits and contributes static @ SMW to the
    output (SMW = sum_h (wv_h wo_h m2_wv m2_wo)[32:, :]); only the 32 glu dims
    participate in scores and the attention-weighted sum.
  - final reshape tiles r 145x, so relu(final) @ out_w1 = relu(r) @ sum_m
    out_w1[m]. The 43MB sum over m is column-sharded 8 ways: each core reduces
    its own 145-column slice fully on-device and a small bf16 AllGather
    (18.5KB/core) assembles the full [64, 1160] on every core.

All arithmetic runs on device (bf16 compute, f32 PSUM accumulation); the host
only marshals layouts (transpose / pad / concat / replicate / constant fill).
"""

import numpy as np
import ml_dtypes

import concourse.bass as bass
import concourse.mybir as mybir
from concourse.bass_utils import run_bass_kernel_spmd
from concourse.tile import TileContext

F32 = mybir.dt.float32
BF16 = mybir.dt.bfloat16
AF = mybir.ActivationFunctionType
ALU = mybir.AluOpType
AX = mybir.AxisListType


def split_multi_waits(nc):
    """The walrus on this image encodes at most ONE sync wait per TPB
    instruction ("Too many sync wait commands" otherwise). Hoist excess waits
    onto standalone InstEventSemaphore ops on the same engine, immediately
    before the instruction - the same mechanism Tile's barriers use."""
    wid = 0
    for f in nc.m.functions:
        for bb in f.blocks:
            out = []
            for ins in bb.instructions:
                si = ins.sync_info
                if si is not None and si.on_wait and len(si.on_wait) > 1:
                    waits = list(si.on_wait)
                    for w in waits[:-1]:
                        wid += 1
                        out.append(mybir.InstEventSemaphore(
                            name=f"Wsplit-{wid}", engine=ins.engine,
                            ins=[], outs=[],
                            sync_info=mybir.SyncInfo(on_wait=[w], on_update=[])))
                    si.on_wait = waits[-1:]
                out.append(ins)
            bb.instructions = out
    return wid


B, T, MED, LAB, GLU, D, H = 1024, 25, 145, 1956, 16, 64, 32
NC_CORES = 8
BC = B // NC_CORES          # 128 patients per core
NH, DH = 4, 16
HID = MED * D // 8          # 1160
CW = HID // NC_CORES        # 145 W1s columns per core

# blob column map (f32 [128, BK]; one on-device bf16 cast, then sliced)
C_WBDG, C_WBDT = 0, 256          # block-diag glu/tf weights    [r0:128]
C_GG, C_GB, C_MG = 512, 544, 576  # glu_gate/glu_b/med_gate rep [r0:128]
C_MWA, C_B1T = 640, 704          # med_w rows 0:128; b1T        [r0:128]
C_MWB = 714                      # med_w rows 128:146           [r0:18]
C_B2 = 778                       # out_b2 row                   [r0:1]
C_WOT, C_M2WVT, C_M2WO = 923, 987, 1051   # [r0:64]
C_SLW2 = 1115                    # sll_w2 + bias row            [r0:65]
C_WQT, C_WKT, C_WVT = 1147, 1403, 1531    # per-head q/k/v^T     [r0:16]
C_GW3G, C_GW3T = 1787, 1819      # visit-24 glu/tf weights      [r0:16]
BK = 1851


def build_bass(split_waits=True):
    nc = bass.Bass()

    def inp(name, shape):
        return nc.dram_tensor(name, list(shape), F32, kind="ExternalInput")

    # ---- per-core inputs (host-marshalled layouts) ----
    w1m_d = inp("w1m", (128, 19, 580))     # out_w1 m-shard, (half,d)-major
    labt_d = inp("labt", (BC, 16, 128))    # lab^T k-tiles (+ones col folded)
    slwt_d = inp("slwt", (BC, 16, D))      # sll_w1 k-tiles (+bias row folded)
    glut_d = inp("glut", (BC, 4, BC))      # glu (j,f)-major transpose
    tft_d = inp("tft", (BC, 4, BC))        # time_feat ditto
    medt_d = inp("medt", (MED + 1, BC))    # med visit-0 transposed + ones row
    w2t_d = inp("w2t", (BC, 10, MED))      # out_w2 k-tiles (1160 rows 0-padded)
    blob_d = inp("blob", (BC, BK))         # packed small weights

    identity = nc.inline_tensor(np.eye(128, dtype=ml_dtypes.bfloat16),
                                name="ident128")

    # collective buffers (DRAM); AllReduce sums the 8 partial W1s
    cc_in = nc.dram_tensor("cc_in", [D, HID], F32)
    cc_out = nc.dram_tensor("cc_out", [D, HID], F32, addr_space="Shared")
    out_d = nc.dram_tensor("out", [BC, MED], F32, kind="ExternalOutput")

    with TileContext(nc) as tc, \
            tc.tile_pool(name="consts", bufs=1) as cp, \
            tc.tile_pool(name="ps", bufs=3, space="PSUM") as ps, \
            tc.tile_pool(name="psg", bufs=1, space="PSUM") as psg, \
            tc.tile_pool(name="pst1", bufs=1, space="PSUM") as pst1, \
            tc.tile_pool(name="pout", bufs=1, space="PSUM") as pout:

        # ================= DMA issue (order = queue execution order) ========
        # Pool/gpsimd queue: the big w1 m-shard, cast f32->bf16 in-flight
        w1m_b = cp.tile([128, 19, 580], BF16, tag="w1m_b")
        nc.gpsimd.dma_start(out=w1m_b[:, 0:10, :], in_=w1m_d[:, 0:10, :])
        nc.gpsimd.dma_start(out=w1m_b[:, 10:19, :], in_=w1m_d[:, 10:19, :])

        # SP queue: blob first (unblocks all weight prep), then big f32 tiles
        blob_f = cp.tile([BC, BK], F32, tag="blob_f")
        nc.sync.dma_start(out=blob_f, in_=blob_d[:])
        labt_f = cp.tile([BC, 16, 128], F32, tag="labt_f")
        nc.sync.dma_start(out=labt_f, in_=labt_d[:])
        slwt_f = cp.tile([BC, 16, D], F32, tag="slwt_f")
        nc.sync.dma_start(out=slwt_f, in_=slwt_d[:])
        w2t_f = cp.tile([BC, 10, MED], F32, tag="w2t_f")
        nc.sync.dma_start(out=w2t_f, in_=w2t_d[:])

        # Act queue: small fast tensors feeding the longest compute chain
        glut_f = cp.tile([BC, 4, BC], F32, tag="glut_f")
        nc.scalar.dma_start(out=glut_f, in_=glut_d[:])
        tft_f = cp.tile([BC, 4, BC], F32, tag="tft_f")
        nc.scalar.dma_start(out=tft_f, in_=tft_d[:])
        med_fa = cp.tile([BC, BC], F32, tag="med_fa")
        nc.scalar.dma_start(out=med_fa, in_=medt_d[0:128, :])
        med_fb = cp.tile([18, BC], F32, tag="med_fb")
        nc.scalar.dma_start(out=med_fb, in_=medt_d[128:146, :])
        ident = cp.tile([128, 128], BF16, tag="ident")
        nc.scalar.dma_start(out=ident, in_=identity[:])

        # ================= bf16 casts (Act engine, readiness order) =========
        glut_b = cp.tile([BC, 4, BC], BF16, tag="glut_b")
        nc.scalar.copy(out=glut_b, in_=glut_f)
        tft_b = cp.tile([BC, 4, BC], BF16, tag="tft_b")
        nc.scalar.copy(out=tft_b, in_=tft_f)
        blob_b = cp.tile([BC, BK], BF16, tag="blob_b")
        nc.scalar.copy(out=blob_b, in_=blob_f)
        labt_b = cp.tile([BC, 16, 128], BF16, tag="labt_b")
        nc.scalar.copy(out=labt_b, in_=labt_f)
        slwt_b = cp.tile([BC, 16, D], BF16, tag="slwt_b")
        nc.scalar.copy(out=slwt_b, in_=slwt_f)

        # ================= med multi-hot (DVE) ==============================
        mb_a = cp.tile([BC, BC], BF16, tag="mb_a")
        nc.vector.tensor_scalar(out=mb_a, in0=med_fa, scalar1=0.9,
                                scalar2=None, op0=ALU.is_gt)
        mb_b = cp.tile([18, BC], BF16, tag="mb_b")
        nc.vector.tensor_scalar(out=mb_b, in0=med_fb, scalar1=0.9,
                                scalar2=None, op0=ALU.is_gt)

        # ================= glu encoder matmuls (PE block-diagonal) ==========
        gx_ps = psg.tile([BC, T, H], F32, tag="gx")
        for c in range(3):
            nc.tensor.matmul(gx_ps[:, 8 * c:8 * c + 8, :],
                             lhsT=glut_b[:, c, :],
                             rhs=blob_b[:, C_WBDG:C_WBDG + 256],
                             start=True, stop=False)
            nc.tensor.matmul(gx_ps[:, 8 * c:8 * c + 8, :],
                             lhsT=tft_b[:, c, :],
                             rhs=blob_b[:, C_WBDT:C_WBDT + 256],
                             start=False, stop=True)
        nc.tensor.matmul(gx_ps[:, 24, :], lhsT=glut_b[0:16, 3, :],
                         rhs=blob_b[0:16, C_GW3G:C_GW3G + 32],
                         start=True, stop=False)
        nc.tensor.matmul(gx_ps[:, 24, :], lhsT=tft_b[0:16, 3, :],
                         rhs=blob_b[0:16, C_GW3T:C_GW3T + 32],
                         start=False, stop=True)

        # med x0 = multihot @ med_w + med_b (bias row folded via ones row)
        x0_ps = ps.tile([BC, D], F32, tag="acc")
        nc.tensor.matmul(x0_ps, lhsT=mb_a,
                         rhs=blob_b[:, C_MWA:C_MWA + D], start=True, stop=False)
        nc.tensor.matmul(x0_ps, lhsT=mb_b,
                         rhs=blob_b[0:18, C_MWB:C_MWB + D], start=False, stop=True)

        # ================= weight prep (PE + gpsimd copies) =================
        # A_h[:, :32] = wq_h wk_h^T / 4 (glu columns only), stacked [64,(h,32)]
        a_ps = ps.tile([D, NH, H], F32, tag="acc")
        for h in range(NH):
            nc.tensor.matmul(a_ps[:, h, :],
                             lhsT=blob_b[0:16, C_WQT + 64 * h:C_WQT + 64 * h + 64],
                             rhs=blob_b[0:16, C_WKT + 32 * h:C_WKT + 32 * h + 32])
        # Wvo2 = m2_wv @ m2_wo
        wvo_ps = ps.tile([D, D], F32, tag="acc")
        nc.tensor.matmul(wvo_ps, lhsT=blob_b[0:D, C_M2WVT:C_M2WVT + D],
                         rhs=blob_b[0:D, C_M2WO:C_M2WO + D])

        a_sb = cp.tile([D, NH, H], BF16, tag="a_sb")
        nc.scalar.activation(out=a_sb, in_=a_ps, func=AF.Copy, scale=0.25)
        wvo_sb = cp.tile([D, D], BF16, tag="wvo_sb")
        nc.scalar.copy(out=wvo_sb, in_=wvo_ps)

        # t_h = wo_h @ Wvo2  [16,(h),64]
        t_ps = ps.tile([16, NH, D], F32, tag="acc")
        for h in range(NH):
            nc.tensor.matmul(t_ps[:, h, :],
                             lhsT=blob_b[0:D, C_WOT + 16 * h:C_WOT + 16 * h + 16],
                             rhs=wvo_sb)
        t_sb = cp.tile([16, NH, D], BF16, tag="t_sb")
        nc.scalar.copy(out=t_sb, in_=t_ps)
        # mwT[e, h, f] = MW_h[f, e] = (wv_h t_h)[f, e], computed transposed
        mwt_ps = ps.tile([D, NH, D], F32, tag="acc")
        for h in range(NH):
            nc.tensor.matmul(mwt_ps[:, h, :], lhsT=t_sb[:, h, :],
                             rhs=blob_b[0:16, C_WVT + 64 * h:C_WVT + 64 * h + 64])
        mwtg_sb = cp.tile([D, NH, H], BF16, tag="mwtg_sb")
        nc.scalar.copy(out=mwtg_sb, in_=mwt_ps[:, :, 0:H])
        mwth_sb = cp.tile([D, NH, H], BF16, tag="mwth_sb")
        nc.scalar.copy(out=mwth_sb, in_=mwt_ps[:, :, H:D])
        # MWg_stack[(h,f<32), e] via one PE transpose of mwT[:, :, :32]
        mwg_ps = ps.tile([128, D], BF16, tag="acc")
        nc.tensor.transpose(mwg_ps, mwtg_sb[:].rearrange("p h f -> p (h f)"),
                            ident[0:D, 0:D])
        mwg_sb = cp.tile([128, D], BF16, tag="mwg_sb")
        nc.scalar.copy(out=mwg_sb, in_=mwg_ps)
        # SMW = sum_h MW_h[32:, :]: sum mwT halves then transpose
        smwt = cp.tile([D, H], BF16, tag="smwt")
        nc.gpsimd.tensor_tensor(out=smwt, in0=mwth_sb[:, 0, :],
                                in1=mwth_sb[:, 1, :], op=ALU.add)
        nc.gpsimd.tensor_tensor(out=smwt, in0=smwt, in1=mwth_sb[:, 2, :],
                                op=ALU.add)
        nc.gpsimd.tensor_tensor(out=smwt, in0=smwt, in1=mwth_sb[:, 3, :],
                                op=ALU.add)
        smw_ps = ps.tile([H, D], BF16, tag="acc")
        nc.tensor.transpose(smw_ps, smwt[:], ident[0:D, 0:D])
        smw_sb = cp.tile([H, D], BF16, tag="smw_sb")
        nc.scalar.copy(out=smw_sb, in_=smw_ps)
        st1rt = cp.tile([D + 1, BC], BF16, tag="st1rt")
        nc.gpsimd.memset(st1rt[D:D + 1, :], 1.0)

        # ================= W1s partial reduce + AllReduce ===================
        # (emitted FIRST on the DVE queue after the multihot ops so the AR
        # input is ready ~34us on every core - the global AllReduce completes
        # when the slowest core contributes)
        def tree(lo, n):  # reduce w1m_b[:, lo:lo+n] into w1m_b[:, lo]
            while n > 1:
                half, odd = n // 2, n % 2
                if odd:
                    nc.vector.tensor_add(w1m_b[:, lo:lo + 1, :],
                                         w1m_b[:, lo:lo + 1, :],
                                         w1m_b[:, lo + n - 1:lo + n, :])
                    n -= 1
                nc.vector.tensor_add(w1m_b[:, lo:lo + half, :],
                                     w1m_b[:, lo:lo + half, :],
                                     w1m_b[:, lo + half:lo + 2 * half, :])
                n = half

        tree(0, 10)
        tree(10, 9)
        w1p = cp.tile([128, 580], BF16, tag="w1p")
        nc.vector.tensor_add(w1p[:].unsqueeze(1), w1m_b[:, 0:1, :],
                             w1m_b[:, 10:11, :])
        w1pf = cp.tile([128, 580], F32, tag="w1pf")
        nc.vector.tensor_copy(out=w1pf, in_=w1p)
        nc.sync.dma_start(
            out=cc_in[:].rearrange("d (r q) -> r d q", r=2), in_=w1pf)
        nc.gpsimd.collective_compute(
            "AllReduce", ALU.add, replica_groups=[list(range(NC_CORES))],
            ins=[cc_in[:]], outs=[cc_out[:]])

        # ================= med gating -> mr0 -> u (critical chain) ==========
        scr = cp.tile([BC, D], BF16, tag="scr")
        nc.vector.tensor_mul(scr, x0_ps, blob_b[:, C_MG:C_MG + D])
        g0 = cp.tile([BC, 1], F32, tag="g0")
        nc.vector.tensor_reduce(out=g0, in_=scr, axis=AX.X, op=ALU.add)
        sg0 = cp.tile([BC, 1], F32, tag="sg0")
        nc.scalar.activation(out=sg0, in_=g0, func=AF.Sigmoid)
        mr0 = cp.tile([BC, D], BF16, tag="mr0")
        nc.vector.tensor_scalar(out=mr0, in0=x0_ps, scalar1=sg0[:, 0:1],
                                scalar2=None, op0=ALU.mult)
        mr0t_ps = ps.tile([D, BC], BF16, tag="acc")
        nc.tensor.transpose(mr0t_ps, mr0[:], ident[:])
        mr0t = cp.tile([D, BC], BF16, tag="mr0t")
        nc.vector.tensor_copy(out=mr0t, in_=mr0t_ps)
        u_ps = ps.tile([BC, NH, H], F32, tag="acc")
        nc.tensor.matmul(u_ps[:].rearrange("p h e -> p (h e)"), lhsT=mr0t,
                         rhs=a_sb[:].rearrange("p h e -> p (h e)"))
        u_sb = cp.tile([BC, NH, H], BF16, tag="u_sb")
        nc.vector.tensor_copy(out=u_sb, in_=u_ps)

        # ================= glu encoder tail (DVE/Act) =======================
        gxb = cp.tile([BC, T, H], BF16, tag="gxb")
        nc.vector.tensor_add(gxb, gx_ps,
                             blob_b[:, C_GB:C_GB + H].unsqueeze(1)
                             .broadcast_to((BC, T, H)))
        grep = cp.tile([BC, T, H], BF16, tag="grep")
        nc.scalar.activation(out=grep, in_=gxb, func=AF.Tanh)
        gm = cp.tile([BC, T, H], BF16, tag="gm")
        nc.vector.tensor_mul(gm, grep,
                             blob_b[:, C_GG:C_GG + H].unsqueeze(1)
                             .broadcast_to((BC, T, H)))
        gs = cp.tile([BC, T], F32, tag="gs")
        nc.vector.tensor_reduce(out=gs, in_=gm, axis=AX.X, op=ALU.add)
        gsg = cp.tile([BC, T], BF16, tag="gsg")
        nc.scalar.activation(out=gsg, in_=gs, func=AF.Sigmoid)
        nc.vector.tensor_mul(grep, grep,
                             gsg[:].unsqueeze(2).broadcast_to((BC, T, H)))

        # ================= one-query attention (glu dims only) ==============
        sprod = cp.tile([BC, NH, T, H], BF16, tag="sprod")
        nc.vector.tensor_mul(
            sprod,
            grep[:].unsqueeze(1).broadcast_to((BC, NH, T, H)),
            u_sb[:].unsqueeze(2).broadcast_to((BC, NH, T, H)))
        # halving-tree reduce over f (keeps fast bf16 DVE throughput)
        for wdt in (16, 8, 4, 2, 1):
            nc.vector.tensor_add(sprod[:, :, :, 0:wdt], sprod[:, :, :, 0:wdt],
                                 sprod[:, :, :, wdt:2 * wdt])
        es = cp.tile([BC, NH, T], BF16, tag="es")
        nc.scalar.activation(out=es, in_=sprod[:, :, :, 0], func=AF.Exp)
        den = cp.tile([BC, NH], F32, tag="den")
        nc.vector.tensor_reduce(out=den, in_=es, axis=AX.X, op=ALU.add)
        rden = cp.tile([BC, NH], F32, tag="rden")
        nc.vector.reciprocal(out=rden, in_=den)
        attn = cp.tile([BC, NH, T], BF16, tag="attn")
        nc.vector.tensor_mul(attn, es,
                             rden[:].unsqueeze(2).broadcast_to((BC, NH, T)))
        # weighted sum over visits, f-major so innermost stays packed
        grept = cp.tile([BC, H, T], BF16, tag="grept")
        nc.vector.tensor_copy(out=grept,
                              in_=grep[:].rearrange("p j f -> p f j"))
        wprod = cp.tile([BC, NH, H, T], BF16, tag="wprod")
        nc.vector.tensor_mul(
            wprod,
            attn[:].unsqueeze(2).broadcast_to((BC, NH, H, T)),
            grept[:].unsqueeze(1).broadcast_to((BC, NH, H, T)))
        nc.vector.tensor_add(wprod[:, :, :, 0:9], wprod[:, :, :, 0:9],
                             wprod[:, :, :, 16:25])
        for wdt in (8, 4, 2, 1):
            nc.vector.tensor_add(wprod[:, :, :, 0:wdt], wprod[:, :, :, 0:wdt],
                                 wprod[:, :, :, wdt:2 * wdt])
        wfin = cp.tile([BC, NH, H], BF16, tag="wfin")
        nc.vector.tensor_copy(out=wfin[:].unsqueeze(3), in_=wprod[:, :, :, 0:1])

        # ================= static MLP (transposed; no lab transpose) ========
        st1_ps = pst1.tile([D, BC], F32, tag="st1")
        for t in range(16):
            nc.tensor.matmul(st1_ps, lhsT=slwt_b[:, t, :], rhs=labt_b[:, t, :],
                             start=(t == 0), stop=(t == 15))
        nc.scalar.activation(out=st1rt[0:D, :], in_=st1_ps, func=AF.Relu)
        stat_ps = ps.tile([H, BC], F32, tag="acc")
        nc.tensor.matmul(stat_ps, lhsT=blob_b[0:D + 1, C_SLW2:C_SLW2 + H],
                         rhs=st1rt)
        statt = cp.tile([H, BC], BF16, tag="statt")
        nc.scalar.activation(out=statt, in_=stat_ps, func=AF.Relu)

        # ================= r = attention out + static part ==================
        wgt_ps = ps.tile([128, BC], BF16, tag="acc")
        nc.tensor.transpose(wgt_ps, wfin[:].rearrange("p h f -> p (h f)"),
                            ident[:])
        wgt_sb = cp.tile([128, BC], BF16, tag="wgt_sb")
        nc.vector.tensor_copy(out=wgt_sb, in_=wgt_ps)
        r_ps = ps.tile([BC, D], F32, tag="acc")
        nc.tensor.matmul(r_ps, lhsT=statt, rhs=smw_sb, start=True, stop=False)
        nc.tensor.matmul(r_ps, lhsT=wgt_sb, rhs=mwg_sb, start=False, stop=True)
        rr = cp.tile([BC, D], BF16, tag="rr")
        nc.scalar.activation(out=rr, in_=r_ps, func=AF.Relu)
        rrt_ps = ps.tile([D, BC], BF16, tag="acc")
        nc.tensor.transpose(rrt_ps, rr[:], ident[:])
        rrt = cp.tile([D, BC], BF16, tag="rrt")
        nc.vector.tensor_copy(out=rrt, in_=rrt_ps)

        # w2 cast late on Act queue (only needed after the AllGather)
        w2t_b = cp.tile([BC, 10, MED], BF16, tag="w2t_b")
        nc.scalar.copy(out=w2t_b, in_=w2t_f)
        ones_sb = cp.tile([1, BC], BF16, tag="ones_sb")
        nc.gpsimd.memset(ones_sb, 1.0)

        # ================= final MLP (after AllGather) ======================
        w1s_f = cp.tile([D, HID], F32, tag="w1s_f")
        nc.sync.dma_start(out=w1s_f, in_=cc_out[:])
        w1s_sb = cp.tile([D, HID], BF16, tag="w1s_sb")
        nc.scalar.copy(out=w1s_sb, in_=w1s_f)
        hidt = cp.tile([128, 10, 128], BF16, tag="hidt")
        # phase 1: all hidT matmuls + relus (pipelined, no out-matmul in between)
        for t in range(10):
            w = 128 if t < 9 else 8
            h_ps = ps.tile([128, BC], F32, tag="acc")
            nc.tensor.matmul(h_ps[0:w, :],
                             lhsT=w1s_sb[:, 128 * t:128 * t + w], rhs=rrt)
            nc.scalar.activation(out=hidt[0:w, t, :], in_=h_ps[0:w, :],
                                 func=AF.Relu,
                                 bias=blob_b[0:w, C_B1T + t:C_B1T + t + 1])
        # phase 2: back-to-back output accumulation
        out_ps = pout.tile([BC, MED], F32, tag="outacc")
        nc.tensor.matmul(out_ps, lhsT=ones_sb,
                         rhs=blob_b[0:1, C_B2:C_B2 + MED],
                         start=True, stop=False, skip_group_check=True)
        for t in range(10):
            w = 128 if t < 9 else 8
            nc.tensor.matmul(out_ps, lhsT=hidt[0:w, t, :],
                             rhs=w2t_b[0:w, t, :],
                             start=False, stop=(t == 9), skip_group_check=True)
        out_sb = cp.tile([BC, MED], F32, tag="out_sb")
        nc.scalar.copy(out=out_sb, in_=out_ps)
        nc.scalar.dma_start(out=out_d[:], in_=out_sb)

    if split_waits:
        split_multi_waits(nc)
    return nc


_CACHED_NC = None


def make_in_maps(inputs):
    """Host-side input marshalling: pure layout work (transpose / pad / concat
    / replicate / constant fill) - every arithmetic op stays on device."""
    f = lambda x: np.ascontiguousarray(np.asarray(x, dtype=np.float32))
    cat = np.concatenate

    # out_w1 [9280, 1160] -> [145, 64, 1160] -> per-core m-shard of 19 blocks,
    # laid out [(half, d), m, 580] so the on-device tree reduce is contiguous
    w1 = f(inputs["out_w1"]).reshape(MED, D, HID)
    w1pad = np.zeros((8 * 19, D, HID), np.float32)
    w1pad[:MED] = w1

    # lab^T k-tiles with ones column folded at row 1957
    lab = f(inputs["lab"])
    lab_ext = np.zeros((B, 2048), np.float32)
    lab_ext[:, :LAB] = lab
    lab_ext[:, LAB] = 1.0           # ones column folds sll_b1 into the matmul

    slw = np.zeros((2048, D), np.float32)
    slw[:LAB] = f(inputs["sll_w1"])
    slw[LAB] = f(inputs["sll_b1"])
    slwt = np.ascontiguousarray(slw.reshape(16, 128, D).transpose(1, 0, 2))

    glu, tf = f(inputs["glu"]), f(inputs["time_feat"])

    def jf_major(x):  # (j, f)-major transpose, padded 400 -> 512
        z = np.zeros((B, 512), np.float32)
        z[:, :T * GLU] = x.reshape(B, T * GLU)
        return z.reshape(B, 4, 128).transpose(2, 1, 0)  # [128p, 4c, B]

    glut, tft = jf_major(glu), jf_major(tf)

    med0 = f(inputs["med"])[:, 0, :]                  # [B, 145]
    medt = np.ones((MED + 1, B), np.float32)          # row 145 = 1.0
    medt[:MED] = med0.T

    w2 = np.zeros((1280, MED), np.float32)
    w2[:HID] = f(inputs["out_w2"])
    w2t = np.ascontiguousarray(w2.reshape(10, 128, MED).transpose(1, 0, 2))

    # ---- packed weight blob ----
    blob = np.zeros((BC, BK), np.float32)
    glu_w = f(inputs["glu_w"])                         # [32, 32]
    for jl in range(8):
        blob[16 * jl:16 * jl + 16,
             C_WBDG + 32 * jl:C_WBDG + 32 * jl + 32] = glu_w[:16]
        blob[16 * jl:16 * jl + 16,
             C_WBDT + 32 * jl:C_WBDT + 32 * jl + 32] = glu_w[16:]
    blob[:, C_GG:C_GG + H] = f(inputs["glu_gate"])[None, :]
    blob[:, C_GB:C_GB + H] = f(inputs["glu_b"])[None, :]
    blob[:, C_MG:C_MG + D] = f(inputs["med_gate"])[None, :]
    medw_ext = cat([f(inputs["med_w"]), f(inputs["med_b"])[None, :]], 0)
    blob[:, C_MWA:C_MWA + D] = medw_ext[:128]
    blob[0:18, C_MWB:C_MWB + D] = medw_ext[128:]
    b1 = f(inputs["out_b1"])
    for t in range(10):
        nvalid = 128 if t < 9 else 8
        blob[:nvalid, C_B1T + t] = b1[128 * t:128 * t + nvalid]
    blob[0, C_B2:C_B2 + MED] = f(inputs["out_b2"])
    wo, wv = f(inputs["m1_wo"]), f(inputs["m1_wv"])
    wq, wk = f(inputs["m1_wq"]), f(inputs["m1_wk"])
    blob[0:D, C_WOT:C_WOT + D] = wo.T                  # woT[d, (h,g)]
    blob[0:D, C_M2WVT:C_M2WVT + D] = f(inputs["m2_wv"]).T
    blob[0:D, C_M2WO:C_M2WO + D] = f(inputs["m2_wo"])
    blob[0:D, C_SLW2:C_SLW2 + H] = f(inputs["sll_w2"])
    blob[D, C_SLW2:C_SLW2 + H] = f(inputs["sll_b2"])
    for h in range(NH):
        blob[0:16, C_WQT + 64 * h:C_WQT + 64 * h + 64] = wq[:, 16 * h:16 * h + 16].T
        blob[0:16, C_WKT + 32 * h:C_WKT + 32 * h + 32] = wk[:H, 16 * h:16 * h + 16].T
        blob[0:16, C_WVT + 64 * h:C_WVT + 64 * h + 64] = wv[:, 16 * h:16 * h + 16].T
    blob[0:16, C_GW3G:C_GW3G + H] = glu_w[:16]
    blob[0:16, C_GW3T:C_GW3T + H] = glu_w[16:]

    in_maps = []
    for c in range(NC_CORES):
        sl = slice(c * BC, (c + 1) * BC)
        in_maps.append({
            "w1m": np.ascontiguousarray(
                w1pad[19 * c:19 * c + 19].reshape(19, D, 2, 580)
                .transpose(2, 1, 0, 3).reshape(128, 19, 580)),
            "labt": np.ascontiguousarray(
                lab_ext[sl].T.reshape(16, 128, BC).transpose(1, 0, 2)),
            "slwt": slwt,
            "glut": np.ascontiguousarray(glut[:, :, sl]),
            "tft": np.ascontiguousarray(tft[:, :, sl]),
            "medt": np.ascontiguousarray(medt[:, sl]),
            "w2t": w2t,
            "blob": blob,
        })
    return in_maps


def kernel(**inputs):
    global _CACHED_NC
    if _CACHED_NC is None:
        _CACHED_NC = build_bass()
    nc = _CACHED_NC
    in_maps = make_in_maps(inputs)
    res = run_bass_kernel_spmd(nc, in_maps, core_ids=list(range(NC_CORES)))
    return np.concatenate([res.results[c]["out"] for c in range(NC_CORES)], axis=0)


if __name__ == "__main__":
    import reference
    inp = reference.setup_inputs()
    out = kernel(**{k: np.asarray(v) for k, v in inp.items()})
    print("kernel output", out.shape, out.dtype)


# revision 12
# speedup vs baseline: 1.1067x; 1.0072x over previous
"""Trainium2 Bass kernel for the MERITS_T patient model (B=1024 data-parallel over 8 cores).

Mathematical simplification of the reference (verified to ~7e-7 rel err in f32,
~4.3e-3 in bf16 against the jax reference; tolerance is 2e-2):
  - E_de = _mha(drug_mem, e0, e0) softmaxes over a single key, so its output is
    e0 @ m2_wv @ m2_wo broadcast over all 145 query rows -> the three GATs, the
    graph MHA and drug_mem never reach the output (dead code).
  - e0 = E_en[:, 0] only needs query row 0 of the m1 attention, i.e. only the
    first visit of `med`.
  - patient_j = [glu_rep_j ; static]: the static half is visit-independent, so
    it cancels in the softmax over visits and contributes static @ SMW to the
    output (SMW = sum_h (wv_h wo_h m2_wv m2_wo)[32:, :]); only the 32 glu dims
    participate in scores and the attention-weighted sum.
  - final reshape tiles r 145x, so relu(final) @ out_w1 = relu(r) @ sum_m
    out_w1[m]. The 43MB sum over m is m-sharded 8 ways: each core tree-reduces
    its 19 blocks in bf16 and one f32 AllReduce (297KB) sums the partials.

All arithmetic runs on device (bf16 compute, f32 PSUM accumulation); the host
only marshals layouts (transpose / pad / concat / replicate / constant fill).
"""

import numpy as np
import ml_dtypes

import concourse.bass as bass
import concourse.mybir as mybir
from concourse.bass_utils import run_bass_kernel_spmd
from concourse.tile import TileContext

F32 = mybir.dt.float32
BF16 = mybir.dt.bfloat16
AF = mybir.ActivationFunctionType
ALU = mybir.AluOpType
AX = mybir.AxisListType


def split_multi_waits(nc):
    """The walrus on this image encodes at most ONE sync wait per TPB
    instruction ("Too many sync wait commands" otherwise). Hoist excess waits
    onto standalone InstEventSemaphore ops on the same engine, immediately
    before the instruction - the same mechanism Tile's barriers use."""
    wid = 0
    for f in nc.m.functions:
        for bb in f.blocks:
            out = []
            for ins in bb.instructions:
                si = ins.sync_info
                if si is not None and si.on_wait and len(si.on_wait) > 1:
                    waits = list(si.on_wait)
                    for w in waits[:-1]:
                        wid += 1
                        out.append(mybir.InstEventSemaphore(
                            name=f"Wsplit-{wid}", engine=ins.engine,
                            ins=[], outs=[],
                            sync_info=mybir.SyncInfo(on_wait=[w], on_update=[])))
                    si.on_wait = waits[-1:]
                out.append(ins)
            bb.instructions = out
    return wid


B, T, MED, LAB, GLU, D, H = 1024, 25, 145, 1956, 16, 64, 32
NC_CORES = 8
BC = B // NC_CORES          # 128 patients per core
NH, DH = 4, 16
HID = MED * D // 8          # 1160
CW = HID // NC_CORES        # 145 W1s columns per core

# blob column map (f32 [128, BK]; one on-device bf16 cast, then sliced)
C_WBDG, C_WBDT = 0, 256          # block-diag glu/tf weights    [r0:128]
C_GG, C_GB, C_MG = 512, 544, 576  # glu_gate/glu_b/med_gate rep [r0:128]
C_MWA, C_B1T = 640, 704          # med_w rows 0:128; b1T        [r0:128]
C_MWB = 714                      # med_w rows 128:146           [r0:18]
C_B2 = 778                       # out_b2 row                   [r0:1]
C_WOT, C_M2WVT, C_M2WO = 923, 987, 1051   # [r0:64]
C_SLW2 = 1115                    # sll_w2 + bias row            [r0:65]
C_WQT, C_WKT, C_WVT = 1147, 1403, 1531    # per-head q/k/v^T     [r0:16]
C_GW3G, C_GW3T = 1787, 1819      # visit-24 glu/tf weights      [r0:16]
BK = 1851


def build_bass(split_waits=True):
    nc = bass.Bass()

    def inp(name, shape):
        return nc.dram_tensor(name, list(shape), F32, kind="ExternalInput")

    # ---- per-core inputs (host-marshalled layouts) ----
    w1m_d = inp("w1m", (128, 19, 580))     # out_w1 m-shard, (half,d)-major
    labt_d = inp("labt", (BC, 16, 128))    # lab^T k-tiles (+ones col folded)
    slwt_d = inp("slwt", (BC, 16, D))      # sll_w1 k-tiles (+bias row folded)
    glut_d = inp("glut", (BC, 4, BC))      # glu (j,f)-major transpose
    tft_d = inp("tft", (BC, 4, BC))        # time_feat ditto
    medt_d = inp("medt", (MED + 1, BC))    # med visit-0 transposed + ones row
    w2t_d = inp("w2t", (BC, 10, MED))      # out_w2 k-tiles (1160 rows 0-padded)
    blob_d = inp("blob", (BC, BK))         # packed small weights

    identity = nc.inline_tensor(np.eye(128, dtype=ml_dtypes.bfloat16),
                                name="ident128")

    # collective buffers (DRAM); AllReduce sums the 8 partial W1s
    cc_in = nc.dram_tensor("cc_in", [D, HID], F32)
    cc_out = nc.dram_tensor("cc_out", [D, HID], F32, addr_space="Shared")
    out_d = nc.dram_tensor("out", [BC, MED], F32, kind="ExternalOutput")

    with TileContext(nc) as tc, \
            tc.tile_pool(name="consts", bufs=1) as cp, \
            tc.tile_pool(name="ps", bufs=3, space="PSUM") as ps, \
            tc.tile_pool(name="psg", bufs=1, space="PSUM") as psg, \
            tc.tile_pool(name="pst1", bufs=1, space="PSUM") as pst1, \
            tc.tile_pool(name="pout", bufs=1, space="PSUM") as pout:

        # ================= DMA issue (order = queue execution order) ========
        # Pool/gpsimd queue: the big w1 m-shard, cast f32->bf16 in-flight
        w1m_b = cp.tile([128, 19, 580], BF16, tag="w1m_b")
        nc.gpsimd.dma_start(out=w1m_b[:, 0:10, :], in_=w1m_d[:, 0:10, :])
        nc.gpsimd.dma_start(out=w1m_b[:, 10:19, :], in_=w1m_d[:, 10:19, :])

        # SP queue: blob first (unblocks all weight prep), then big f32 tiles
        blob_f = cp.tile([BC, BK], F32, tag="blob_f")
        nc.sync.dma_start(out=blob_f, in_=blob_d[:])
        labt_f = cp.tile([BC, 16, 128], F32, tag="labt_f")
        nc.sync.dma_start(out=labt_f, in_=labt_d[:])
        slwt_f = cp.tile([BC, 16, D], F32, tag="slwt_f")
        nc.sync.dma_start(out=slwt_f, in_=slwt_d[:])
        w2t_f = cp.tile([BC, 10, MED], F32, tag="w2t_f")
        nc.sync.dma_start(out=w2t_f, in_=w2t_d[:])

        # Act queue: small fast tensors feeding the longest compute chain
        glut_f = cp.tile([BC, 4, BC], F32, tag="glut_f")
        nc.scalar.dma_start(out=glut_f, in_=glut_d[:])
        tft_f = cp.tile([BC, 4, BC], F32, tag="tft_f")
        nc.scalar.dma_start(out=tft_f, in_=tft_d[:])
        med_fa = cp.tile([BC, BC], F32, tag="med_fa")
        nc.scalar.dma_start(out=med_fa, in_=medt_d[0:128, :])
        med_fb = cp.tile([18, BC], F32, tag="med_fb")
        nc.scalar.dma_start(out=med_fb, in_=medt_d[128:146, :])
        ident = cp.tile([128, 128], BF16, tag="ident")
        nc.scalar.dma_start(out=ident, in_=identity[:])

        # ================= bf16 casts (Act engine, readiness order) =========
        glut_b = cp.tile([BC, 4, BC], BF16, tag="glut_b")
        nc.scalar.copy(out=glut_b, in_=glut_f)
        tft_b = cp.tile([BC, 4, BC], BF16, tag="tft_b")
        nc.scalar.copy(out=tft_b, in_=tft_f)
        blob_b = cp.tile([BC, BK], BF16, tag="blob_b")
        nc.scalar.copy(out=blob_b, in_=blob_f)
        labt_b = cp.tile([BC, 16, 128], BF16, tag="labt_b")
        nc.scalar.copy(out=labt_b, in_=labt_f)
        slwt_b = cp.tile([BC, 16, D], BF16, tag="slwt_b")
        nc.scalar.copy(out=slwt_b, in_=slwt_f)

        # ================= med multi-hot (DVE) ==============================
        mb_a = cp.tile([BC, BC], BF16, tag="mb_a")
        nc.vector.tensor_scalar(out=mb_a, in0=med_fa, scalar1=0.9,
                                scalar2=None, op0=ALU.is_gt)
        mb_b = cp.tile([18, BC], BF16, tag="mb_b")
        nc.vector.tensor_scalar(out=mb_b, in0=med_fb, scalar1=0.9,
                                scalar2=None, op0=ALU.is_gt)

        # ================= glu encoder matmuls (PE block-diagonal) ==========
        gx_ps = psg.tile([BC, T, H], F32, tag="gx")
        for c in range(3):
            nc.tensor.matmul(gx_ps[:, 8 * c:8 * c + 8, :],
                             lhsT=glut_b[:, c, :],
                             rhs=blob_b[:, C_WBDG:C_WBDG + 256],
                             start=True, stop=False)
            nc.tensor.matmul(gx_ps[:, 8 * c:8 * c + 8, :],
                             lhsT=tft_b[:, c, :],
                             rhs=blob_b[:, C_WBDT:C_WBDT + 256],
                             start=False, stop=True)
        nc.tensor.matmul(gx_ps[:, 24, :], lhsT=glut_b[0:16, 3, :],
                         rhs=blob_b[0:16, C_GW3G:C_GW3G + 32],
                         start=True, stop=False)
        nc.tensor.matmul(gx_ps[:, 24, :], lhsT=tft_b[0:16, 3, :],
                         rhs=blob_b[0:16, C_GW3T:C_GW3T + 32],
                         start=False, stop=True)

        # med x0 = multihot @ med_w + med_b (bias row folded via ones row)
        x0_ps = ps.tile([BC, D], F32, tag="acc")
        nc.tensor.matmul(x0_ps, lhsT=mb_a,
                         rhs=blob_b[:, C_MWA:C_MWA + D], start=True, stop=False)
        nc.tensor.matmul(x0_ps, lhsT=mb_b,
                         rhs=blob_b[0:18, C_MWB:C_MWB + D], start=False, stop=True)

        # ================= weight prep (PE + gpsimd copies) =================
        # A_h[:, :32] = wq_h wk_h^T / 4 (glu columns only), stacked [64,(h,32)]
        a_ps = ps.tile([D, NH, H], F32, tag="acc")
        for h in range(NH):
            nc.tensor.matmul(a_ps[:, h, :],
                             lhsT=blob_b[0:16, C_WQT + 64 * h:C_WQT + 64 * h + 64],
                             rhs=blob_b[0:16, C_WKT + 32 * h:C_WKT + 32 * h + 32])
        # Wvo2 = m2_wv @ m2_wo
        wvo_ps = ps.tile([D, D], F32, tag="acc")
        nc.tensor.matmul(wvo_ps, lhsT=blob_b[0:D, C_M2WVT:C_M2WVT + D],
                         rhs=blob_b[0:D, C_M2WO:C_M2WO + D])

        a_sb = cp.tile([D, NH, H], BF16, tag="a_sb")
        nc.scalar.activation(out=a_sb, in_=a_ps, func=AF.Copy, scale=0.25)
        wvo_sb = cp.tile([D, D], BF16, tag="wvo_sb")
        nc.scalar.copy(out=wvo_sb, in_=wvo_ps)

        # t_h = wo_h @ Wvo2  [16,(h),64]
        t_ps = ps.tile([16, NH, D], F32, tag="acc")
        for h in range(NH):
            nc.tensor.matmul(t_ps[:, h, :],
                             lhsT=blob_b[0:D, C_WOT + 16 * h:C_WOT + 16 * h + 16],
                             rhs=wvo_sb)
        t_sb = cp.tile([16, NH, D], BF16, tag="t_sb")
        nc.scalar.copy(out=t_sb, in_=t_ps)
        # mwT[e, h, f] = MW_h[f, e] = (wv_h t_h)[f, e], computed transposed
        mwt_ps = ps.tile([D, NH, D], F32, tag="acc")
        for h in range(NH):
            nc.tensor.matmul(mwt_ps[:, h, :], lhsT=t_sb[:, h, :],
                             rhs=blob_b[0:16, C_WVT + 64 * h:C_WVT + 64 * h + 64])
        mwtg_sb = cp.tile([D, NH, H], BF16, tag="mwtg_sb")
        nc.scalar.copy(out=mwtg_sb, in_=mwt_ps[:, :, 0:H])
        mwth_sb = cp.tile([D, NH, H], BF16, tag="mwth_sb")
        nc.scalar.copy(out=mwth_sb, in_=mwt_ps[:, :, H:D])
        # MWg_stack[(h,f<32), e] via one PE transpose of mwT[:, :, :32]
        mwg_ps = ps.tile([128, D], BF16, tag="acc")
        nc.tensor.transpose(mwg_ps, mwtg_sb[:].rearrange("p h f -> p (h f)"),
                            ident[0:D, 0:D])
        mwg_sb = cp.tile([128, D], BF16, tag="mwg_sb")
        nc.scalar.copy(out=mwg_sb, in_=mwg_ps)
        # SMW = sum_h MW_h[32:, :]: sum mwT halves then transpose
        smwt = cp.tile([D, H], BF16, tag="smwt")
        nc.gpsimd.tensor_tensor(out=smwt, in0=mwth_sb[:, 0, :],
                                in1=mwth_sb[:, 1, :], op=ALU.add)
        nc.gpsimd.tensor_tensor(out=smwt, in0=smwt, in1=mwth_sb[:, 2, :],
                                op=ALU.add)
        nc.gpsimd.tensor_tensor(out=smwt, in0=smwt, in1=mwth_sb[:, 3, :],
                                op=ALU.add)
        smw_ps = ps.tile([H, D], BF16, tag="acc")
        nc.tensor.transpose(smw_ps, smwt[:], ident[0:D, 0:D])
        smw_sb = cp.tile([H, D], BF16, tag="smw_sb")
        nc.scalar.copy(out=smw_sb, in_=smw_ps)
        st1rt = cp.tile([D + 1, BC], BF16, tag="st1rt")
        nc.gpsimd.memset(st1rt[D:D + 1, :], 1.0)

        # ================= W1s partial reduce + AllReduce ===================
        # (emitted FIRST on the DVE queue after the multihot ops so the AR
        # input is ready ~34us on every core - the global AllReduce completes
        # when the slowest core contributes)
        def tree(lo, n):  # reduce w1m_b[:, lo:lo+n] into w1m_b[:, lo]
            while n > 1:
                half, odd = n // 2, n % 2
                if odd:
                    nc.vector.tensor_add(w1m_b[:, lo:lo + 1, :],
                                         w1m_b[:, lo:lo + 1, :],
                                         w1m_b[:, lo + n - 1:lo + n, :])
                    n -= 1
                nc.vector.tensor_add(w1m_b[:, lo:lo + half, :],
                                     w1m_b[:, lo:lo + half, :],
                                     w1m_b[:, lo + half:lo + 2 * half, :])
                n = half

        tree(0, 10)
        tree(10, 9)
        w1p = cp.tile([128, 580], BF16, tag="w1p")
        nc.vector.tensor_add(w1p[:].unsqueeze(1), w1m_b[:, 0:1, :],
                             w1m_b[:, 10:11, :])
        w1pf = cp.tile([128, 580], F32, tag="w1pf")
        nc.vector.tensor_copy(out=w1pf, in_=w1p)
        nc.sync.dma_start(
            out=cc_in[:].rearrange("d (r q) -> r d q", r=2), in_=w1pf)
        nc.gpsimd.collective_compute(
            "AllReduce", ALU.add, replica_groups=[list(range(NC_CORES))],
            ins=[cc_in[:]], outs=[cc_out[:]])

        # ================= med gating -> mr0 -> u (critical chain) ==========
        scr = cp.tile([BC, D], BF16, tag="scr")
        nc.vector.tensor_mul(scr, x0_ps, blob_b[:, C_MG:C_MG + D])
        g0 = cp.tile([BC, 1], F32, tag="g0")
        nc.vector.tensor_reduce(out=g0, in_=scr, axis=AX.X, op=ALU.add)
        sg0 = cp.tile([BC, 1], F32, tag="sg0")
        nc.scalar.activation(out=sg0, in_=g0, func=AF.Sigmoid)
        mr0 = cp.tile([BC, D], BF16, tag="mr0")
        nc.vector.tensor_scalar(out=mr0, in0=x0_ps, scalar1=sg0[:, 0:1],
                                scalar2=None, op0=ALU.mult)
        mr0t_ps = ps.tile([D, BC], BF16, tag="acc")
        nc.tensor.transpose(mr0t_ps, mr0[:], ident[:])
        mr0t = cp.tile([D, BC], BF16, tag="mr0t")
        nc.vector.tensor_copy(out=mr0t, in_=mr0t_ps)
        u_ps = ps.tile([BC, NH, H], F32, tag="acc")
        nc.tensor.matmul(u_ps[:].rearrange("p h e -> p (h e)"), lhsT=mr0t,
                         rhs=a_sb[:].rearrange("p h e -> p (h e)"))
        u_sb = cp.tile([BC, NH, H], BF16, tag="u_sb")
        nc.vector.tensor_copy(out=u_sb, in_=u_ps)

        # ================= glu encoder tail (DVE/Act) =======================
        gxb = cp.tile([BC, T, H], BF16, tag="gxb")
        nc.vector.tensor_add(gxb, gx_ps,
                             blob_b[:, C_GB:C_GB + H].unsqueeze(1)
                             .broadcast_to((BC, T, H)))
        grep = cp.tile([BC, T, H], BF16, tag="grep")
        nc.scalar.activation(out=grep, in_=gxb, func=AF.Tanh)
        gm = cp.tile([BC, T, H], BF16, tag="gm")
        nc.vector.tensor_mul(gm, grep,
                             blob_b[:, C_GG:C_GG + H].unsqueeze(1)
                             .broadcast_to((BC, T, H)))
        gs = cp.tile([BC, T], F32, tag="gs")
        nc.vector.tensor_reduce(out=gs, in_=gm, axis=AX.X, op=ALU.add)
        gsg = cp.tile([BC, T], BF16, tag="gsg")
        nc.scalar.activation(out=gsg, in_=gs, func=AF.Sigmoid)
        nc.vector.tensor_mul(grep, grep,
                             gsg[:].unsqueeze(2).broadcast_to((BC, T, H)))

        # ================= one-query attention (glu dims only) ==============
        sprod = cp.tile([BC, NH, T, H], BF16, tag="sprod")
        nc.vector.tensor_mul(
            sprod,
            grep[:].unsqueeze(1).broadcast_to((BC, NH, T, H)),
            u_sb[:].unsqueeze(2).broadcast_to((BC, NH, T, H)))
        # halving-tree reduce over f (keeps fast bf16 DVE throughput)
        for wdt in (16, 8, 4, 2, 1):
            nc.vector.tensor_add(sprod[:, :, :, 0:wdt], sprod[:, :, :, 0:wdt],
                                 sprod[:, :, :, wdt:2 * wdt])
        es = cp.tile([BC, NH, T], BF16, tag="es")
        nc.scalar.activation(out=es, in_=sprod[:, :, :, 0], func=AF.Exp)
        den = cp.tile([BC, NH], F32, tag="den")
        nc.vector.tensor_reduce(out=den, in_=es, axis=AX.X, op=ALU.add)
        rden = cp.tile([BC, NH], F32, tag="rden")
        nc.vector.reciprocal(out=rden, in_=den)
        attn = cp.tile([BC, NH, T], BF16, tag="attn")
        nc.vector.tensor_mul(attn, es,
                             rden[:].unsqueeze(2).broadcast_to((BC, NH, T)))
        # weighted sum over visits, f-major so innermost stays packed
        grept = cp.tile([BC, H, T], BF16, tag="grept")
        nc.vector.tensor_copy(out=grept,
                              in_=grep[:].rearrange("p j f -> p f j"))
        wprod = cp.tile([BC, NH, H, T], BF16, tag="wprod")
        nc.vector.tensor_mul(
            wprod,
            attn[:].unsqueeze(2).broadcast_to((BC, NH, H, T)),
            grept[:].unsqueeze(1).broadcast_to((BC, NH, H, T)))
        nc.vector.tensor_add(wprod[:, :, :, 0:9], wprod[:, :, :, 0:9],
                             wprod[:, :, :, 16:25])
        for wdt in (8, 4, 2, 1):
            nc.vector.tensor_add(wprod[:, :, :, 0:wdt], wprod[:, :, :, 0:wdt],
                                 wprod[:, :, :, wdt:2 * wdt])
        wfin = cp.tile([BC, NH, H], BF16, tag="wfin")
        nc.vector.tensor_copy(out=wfin[:].unsqueeze(3), in_=wprod[:, :, :, 0:1])

        # ================= static MLP (transposed; no lab transpose) ========
        st1_ps = pst1.tile([D, BC], F32, tag="st1")
        for t in range(16):
            nc.tensor.matmul(st1_ps, lhsT=slwt_b[:, t, :], rhs=labt_b[:, t, :],
                             start=(t == 0), stop=(t == 15))
        nc.scalar.activation(out=st1rt[0:D, :], in_=st1_ps, func=AF.Relu)
        stat_ps = ps.tile([H, BC], F32, tag="acc")
        nc.tensor.matmul(stat_ps, lhsT=blob_b[0:D + 1, C_SLW2:C_SLW2 + H],
                         rhs=st1rt)
        statt = cp.tile([H, BC], BF16, tag="statt")
        nc.scalar.activation(out=statt, in_=stat_ps, func=AF.Relu)

        # ================= r = attention out + static part ==================
        wgt_ps = ps.tile([128, BC], BF16, tag="acc")
        nc.tensor.transpose(wgt_ps, wfin[:].rearrange("p h f -> p (h f)"),
                            ident[:])
        wgt_sb = cp.tile([128, BC], BF16, tag="wgt_sb")
        nc.vector.tensor_copy(out=wgt_sb, in_=wgt_ps)
        r_ps = ps.tile([BC, D], F32, tag="acc")
        nc.tensor.matmul(r_ps, lhsT=statt, rhs=smw_sb, start=True, stop=False)
        nc.tensor.matmul(r_ps, lhsT=wgt_sb, rhs=mwg_sb, start=False, stop=True)
        rr = cp.tile([BC, D], BF16, tag="rr")
        nc.scalar.activation(out=rr, in_=r_ps, func=AF.Relu)
        rrt_ps = ps.tile([D, BC], BF16, tag="acc")
        nc.tensor.transpose(rrt_ps, rr[:], ident[:])
        rrt = cp.tile([D, BC], BF16, tag="rrt")
        nc.vector.tensor_copy(out=rrt, in_=rrt_ps)

        # w2 cast late on Act queue (only needed after the AllGather)
        w2t_b = cp.tile([BC, 10, MED], BF16, tag="w2t_b")
        nc.scalar.copy(out=w2t_b, in_=w2t_f)
        ones_sb = cp.tile([1, BC], BF16, tag="ones_sb")
        nc.gpsimd.memset(ones_sb, 1.0)

        # ================= final MLP (after AllGather) ======================
        # chunked readback: each 512-col chunk lands, casts, and feeds its
        # hidT matmuls while later chunks are still in flight
        w1s_f = cp.tile([D, HID], F32, tag="w1s_f")
        w1s_sb = cp.tile([D, HID], BF16, tag="w1s_sb")
        for o, n in ((0, 512), (512, 512), (1024, 136)):
            nc.sync.dma_start(out=w1s_f[:, o:o + n], in_=cc_out[:, o:o + n])
            nc.scalar.copy(out=w1s_sb[:, o:o + n], in_=w1s_f[:, o:o + n])
        hidt = cp.tile([128, 10, 128], BF16, tag="hidt")
        # phase 1: all hidT matmuls + relus (pipelined, no out-matmul in between)
        for t in range(10):
            w = 128 if t < 9 else 8
            h_ps = ps.tile([128, BC], F32, tag="acc")
            nc.tensor.matmul(h_ps[0:w, :],
                             lhsT=w1s_sb[:, 128 * t:128 * t + w], rhs=rrt)
            nc.scalar.activation(out=hidt[0:w, t, :], in_=h_ps[0:w, :],
                                 func=AF.Relu,
                                 bias=blob_b[0:w, C_B1T + t:C_B1T + t + 1])
        # phase 2: back-to-back output accumulation
        out_ps = pout.tile([BC, MED], F32, tag="outacc")
        nc.tensor.matmul(out_ps, lhsT=ones_sb,
                         rhs=blob_b[0:1, C_B2:C_B2 + MED],
                         start=True, stop=False, skip_group_check=True)
        for t in range(10):
            w = 128 if t < 9 else 8
            nc.tensor.matmul(out_ps, lhsT=hidt[0:w, t, :],
                             rhs=w2t_b[0:w, t, :],
                             start=False, stop=(t == 9), skip_group_check=True)
        out_sb = cp.tile([BC, MED], F32, tag="out_sb")
        nc.scalar.copy(out=out_sb, in_=out_ps)
        nc.scalar.dma_start(out=out_d[:], in_=out_sb)

    if split_waits:
        split_multi_waits(nc)
    return nc


_CACHED_NC = None


def make_in_maps(inputs):
    """Host-side input marshalling: pure layout work (transpose / pad / concat
    / replicate / constant fill) - every arithmetic op stays on device."""
    f = lambda x: np.ascontiguousarray(np.asarray(x, dtype=np.float32))
    cat = np.concatenate

    # out_w1 [9280, 1160] -> [145, 64, 1160] -> per-core m-shard of 19 blocks,
    # laid out [(half, d), m, 580] so the on-device tree reduce is contiguous
    w1 = f(inputs["out_w1"]).reshape(MED, D, HID)
    w1pad = np.zeros((8 * 19, D, HID), np.float32)
    w1pad[:MED] = w1

    # lab^T k-tiles with ones column folded at row 1957
    lab = f(inputs["lab"])
    lab_ext = np.zeros((B, 2048), np.float32)
    lab_ext[:, :LAB] = lab
    lab_ext[:, LAB] = 1.0           # ones column folds sll_b1 into the matmul

    slw = np.zeros((2048, D), np.float32)
    slw[:LAB] = f(inputs["sll_w1"])
    slw[LAB] = f(inputs["sll_b1"])
    slwt = np.ascontiguousarray(slw.reshape(16, 128, D).transpose(1, 0, 2))

    glu, tf = f(inputs["glu"]), f(inputs["time_feat"])

    def jf_major(x):  # (j, f)-major transpose, padded 400 -> 512
        z = np.zeros((B, 512), np.float32)
        z[:, :T * GLU] = x.reshape(B, T * GLU)
        return z.reshape(B, 4, 128).transpose(2, 1, 0)  # [128p, 4c, B]

    glut, tft = jf_major(glu), jf_major(tf)

    med0 = f(inputs["med"])[:, 0, :]                  # [B, 145]
    medt = np.ones((MED + 1, B), np.float32)          # row 145 = 1.0
    medt[:MED] = med0.T

    w2 = np.zeros((1280, MED), np.float32)
    w2[:HID] = f(inputs["out_w2"])
    w2t = np.ascontiguousarray(w2.reshape(10, 128, MED).transpose(1, 0, 2))

    # ---- packed weight blob ----
    blob = np.zeros((BC, BK), np.float32)
    glu_w = f(inputs["glu_w"])                         # [32, 32]
    for jl in range(8):
        blob[16 * jl:16 * jl + 16,
             C_WBDG + 32 * jl:C_WBDG + 32 * jl + 32] = glu_w[:16]
        blob[16 * jl:16 * jl + 16,
             C_WBDT + 32 * jl:C_WBDT + 32 * jl + 32] = glu_w[16:]
    blob[:, C_GG:C_GG + H] = f(inputs["glu_gate"])[None, :]
    blob[:, C_GB:C_GB + H] = f(inputs["glu_b"])[None, :]
    blob[:, C_MG:C_MG + D] = f(inputs["med_gate"])[None, :]
    medw_ext = cat([f(inputs["med_w"]), f(inputs["med_b"])[None, :]], 0)
    blob[:, C_MWA:C_MWA + D] = medw_ext[:128]
    blob[0:18, C_MWB:C_MWB + D] = medw_ext[128:]
    b1 = f(inputs["out_b1"])
    for t in range(10):
        nvalid = 128 if t < 9 else 8
        blob[:nvalid, C_B1T + t] = b1[128 * t:128 * t + nvalid]
    blob[0, C_B2:C_B2 + MED] = f(inputs["out_b2"])
    wo, wv = f(inputs["m1_wo"]), f(inputs["m1_wv"])
    wq, wk = f(inputs["m1_wq"]), f(inputs["m1_wk"])
    blob[0:D, C_WOT:C_WOT + D] = wo.T                  # woT[d, (h,g)]
    blob[0:D, C_M2WVT:C_M2WVT + D] = f(inputs["m2_wv"]).T
    blob[0:D, C_M2WO:C_M2WO + D] = f(inputs["m2_wo"])
    blob[0:D, C_SLW2:C_SLW2 + H] = f(inputs["sll_w2"])
    blob[D, C_SLW2:C_SLW2 + H] = f(inputs["sll_b2"])
    for h in range(NH):
        blob[0:16, C_WQT + 64 * h:C_WQT + 64 * h + 64] = wq[:, 16 * h:16 * h + 16].T
        blob[0:16, C_WKT + 32 * h:C_WKT + 32 * h + 32] = wk[:H, 16 * h:16 * h + 16].T
        blob[0:16, C_WVT + 64 * h:C_WVT + 64 * h + 64] = wv[:, 16 * h:16 * h + 16].T
    blob[0:16, C_GW3G:C_GW3G + H] = glu_w[:16]
    blob[0:16, C_GW3T:C_GW3T + H] = glu_w[16:]

    in_maps = []
    for c in range(NC_CORES):
        sl = slice(c * BC, (c + 1) * BC)
        in_maps.append({
            "w1m": np.ascontiguousarray(
                w1pad[19 * c:19 * c + 19].reshape(19, D, 2, 580)
                .transpose(2, 1, 0, 3).reshape(128, 19, 580)),
            "labt": np.ascontiguousarray(
                lab_ext[sl].T.reshape(16, 128, BC).transpose(1, 0, 2)),
            "slwt": slwt,
            "glut": np.ascontiguousarray(glut[:, :, sl]),
            "tft": np.ascontiguousarray(tft[:, :, sl]),
            "medt": np.ascontiguousarray(medt[:, sl]),
            "w2t": w2t,
            "blob": blob,
        })
    return in_maps


def kernel(**inputs):
    global _CACHED_NC
    if _CACHED_NC is None:
        _CACHED_NC = build_bass()
    nc = _CACHED_NC
    in_maps = make_in_maps(inputs)
    res = run_bass_kernel_spmd(nc, in_maps, core_ids=list(range(NC_CORES)))
    return np.concatenate([res.results[c]["out"] for c in range(NC_CORES)], axis=0)


if __name__ == "__main__":
    import reference
    inp = reference.setup_inputs()
    out = kernel(**{k: np.asarray(v) for k, v in inp.items()})
    print("kernel output", out.shape, out.dtype)


# revision 14
# speedup vs baseline: 1.1144x; 1.0069x over previous
"""Trainium2 Bass kernel for the MERITS_T patient model (B=1024 data-parallel over 8 cores).

Mathematical simplification of the reference (verified to ~7e-7 rel err in f32,
~4.3e-3 in bf16 against the jax reference; tolerance is 2e-2):
  - E_de = _mha(drug_mem, e0, e0) softmaxes over a single key, so its output is
    e0 @ m2_wv @ m2_wo broadcast over all 145 query rows -> the three GATs, the
    graph MHA and drug_mem never reach the output (dead code).
  - e0 = E_en[:, 0] only needs query row 0 of the m1 attention, i.e. only the
    first visit of `med`.
  - patient_j = [glu_rep_j ; static]: the static half is visit-independent, so
    it cancels in the softmax over visits and contributes static @ SMW to the
    output (SMW = sum_h (wv_h wo_h m2_wv m2_wo)[32:, :]); only the 32 glu dims
    participate in scores and the attention-weighted sum.
  - final reshape tiles r 145x, so relu(final) @ out_w1 = relu(r) @ sum_m
    out_w1[m]. The 43MB sum over m is m-sharded 8 ways: each core tree-reduces
    its 19 blocks in bf16 and one f32 AllReduce (297KB) sums the partials.

All arithmetic runs on device (bf16 compute, f32 PSUM accumulation); the host
only marshals layouts (transpose / pad / concat / replicate / constant fill).
"""

import numpy as np
import ml_dtypes

import concourse.bass as bass
import concourse.mybir as mybir
from concourse.bass_utils import run_bass_kernel_spmd
from concourse.tile import TileContext

F32 = mybir.dt.float32
BF16 = mybir.dt.bfloat16
AF = mybir.ActivationFunctionType
ALU = mybir.AluOpType
AX = mybir.AxisListType


def split_multi_waits(nc):
    """The walrus on this image encodes at most ONE sync wait per TPB
    instruction ("Too many sync wait commands" otherwise). Hoist excess waits
    onto standalone InstEventSemaphore ops on the same engine, immediately
    before the instruction - the same mechanism Tile's barriers use."""
    wid = 0
    for f in nc.m.functions:
        for bb in f.blocks:
            out = []
            for ins in bb.instructions:
                si = ins.sync_info
                if si is not None and si.on_wait and len(si.on_wait) > 1:
                    waits = list(si.on_wait)
                    for w in waits[:-1]:
                        wid += 1
                        out.append(mybir.InstEventSemaphore(
                            name=f"Wsplit-{wid}", engine=ins.engine,
                            ins=[], outs=[],
                            sync_info=mybir.SyncInfo(on_wait=[w], on_update=[])))
                    si.on_wait = waits[-1:]
                out.append(ins)
            bb.instructions = out
    return wid


B, T, MED, LAB, GLU, D, H = 1024, 25, 145, 1956, 16, 64, 32
NC_CORES = 8
BC = B // NC_CORES          # 128 patients per core
NH, DH = 4, 16
HID = MED * D // 8          # 1160
CW = HID // NC_CORES        # 145 W1s columns per core

# blob column map (f32 [128, BK]; one on-device bf16 cast, then sliced)
C_WBDG, C_WBDT = 0, 256          # block-diag glu/tf weights    [r0:128]
C_GG, C_GB, C_MG = 512, 544, 576  # glu_gate/glu_b/med_gate rep [r0:128]
C_MWA, C_B1T = 640, 704          # med_w rows 0:128; b1T        [r0:128]
C_MWB = 714                      # med_w rows 128:146           [r0:18]
C_B2 = 778                       # out_b2 row                   [r0:1]
C_WOT, C_M2WVT, C_M2WO = 923, 987, 1051   # [r0:64]
C_SLW2 = 1115                    # sll_w2 + bias row            [r0:65]
C_WQT, C_WKT, C_WVT = 1147, 1403, 1531    # per-head q/k/v^T     [r0:16]
C_GW3G, C_GW3T = 1787, 1819      # visit-24 glu/tf weights      [r0:16]
BK = 1851


def build_bass(split_waits=True):
    nc = bass.Bass()

    def inp(name, shape):
        return nc.dram_tensor(name, list(shape), F32, kind="ExternalInput")

    # ---- per-core inputs (host-marshalled layouts) ----
    w1m_d = inp("w1m", (128, 19, 580))     # out_w1 m-shard, (half,d)-major
    labt_d = inp("labt", (BC, 16, 128))    # lab^T k-tiles (+ones col folded)
    slwt_d = inp("slwt", (BC, 16, D))      # sll_w1 k-tiles (+bias row folded)
    glut_d = inp("glut", (BC, 4, BC))      # glu (j,f)-major transpose
    tft_d = inp("tft", (BC, 4, BC))        # time_feat ditto
    medt_d = inp("medt", (MED + 1, BC))    # med visit-0 transposed + ones row
    w2t_d = inp("w2t", (BC, 10, MED))      # out_w2 k-tiles (1160 rows 0-padded)
    blob_d = inp("blob", (BC, BK))         # packed small weights

    identity = nc.inline_tensor(np.eye(128, dtype=ml_dtypes.bfloat16),
                                name="ident128")

    # collective buffers (DRAM); bf16 AllReduce sums the 8 partial W1s
    cc_in = nc.dram_tensor("cc_in", [D, HID], BF16)
    cc_out = nc.dram_tensor("cc_out", [D, HID], BF16, addr_space="Shared")
    out_d = nc.dram_tensor("out", [BC, MED], F32, kind="ExternalOutput")

    with TileContext(nc) as tc, \
            tc.tile_pool(name="consts", bufs=1) as cp, \
            tc.tile_pool(name="ps", bufs=3, space="PSUM") as ps, \
            tc.tile_pool(name="psg", bufs=1, space="PSUM") as psg, \
            tc.tile_pool(name="pst1", bufs=1, space="PSUM") as pst1, \
            tc.tile_pool(name="pout", bufs=1, space="PSUM") as pout:

        # ================= DMA issue (order = queue execution order) ========
        # Pool/gpsimd queue: the big w1 m-shard, cast f32->bf16 in-flight
        w1m_b = cp.tile([128, 19, 580], BF16, tag="w1m_b")
        nc.gpsimd.dma_start(out=w1m_b[:, 0:10, :], in_=w1m_d[:, 0:10, :])
        nc.gpsimd.dma_start(out=w1m_b[:, 10:19, :], in_=w1m_d[:, 10:19, :])

        # SP queue: blob first (unblocks all weight prep), then big f32 tiles
        blob_f = cp.tile([BC, BK], F32, tag="blob_f")
        nc.sync.dma_start(out=blob_f, in_=blob_d[:])
        labt_f = cp.tile([BC, 16, 128], F32, tag="labt_f")
        nc.sync.dma_start(out=labt_f, in_=labt_d[:])
        slwt_f = cp.tile([BC, 16, D], F32, tag="slwt_f")
        nc.sync.dma_start(out=slwt_f, in_=slwt_d[:])
        w2t_f = cp.tile([BC, 10, MED], F32, tag="w2t_f")
        nc.sync.dma_start(out=w2t_f, in_=w2t_d[:])

        # Act queue: small fast tensors feeding the longest compute chain
        glut_f = cp.tile([BC, 4, BC], F32, tag="glut_f")
        nc.scalar.dma_start(out=glut_f, in_=glut_d[:])
        tft_f = cp.tile([BC, 4, BC], F32, tag="tft_f")
        nc.scalar.dma_start(out=tft_f, in_=tft_d[:])
        med_fa = cp.tile([BC, BC], F32, tag="med_fa")
        nc.scalar.dma_start(out=med_fa, in_=medt_d[0:128, :])
        med_fb = cp.tile([18, BC], F32, tag="med_fb")
        nc.scalar.dma_start(out=med_fb, in_=medt_d[128:146, :])
        ident = cp.tile([128, 128], BF16, tag="ident")
        nc.scalar.dma_start(out=ident, in_=identity[:])

        # ================= bf16 casts (Act engine, readiness order) =========
        glut_b = cp.tile([BC, 4, BC], BF16, tag="glut_b")
        nc.scalar.copy(out=glut_b, in_=glut_f)
        tft_b = cp.tile([BC, 4, BC], BF16, tag="tft_b")
        nc.scalar.copy(out=tft_b, in_=tft_f)
        blob_b = cp.tile([BC, BK], BF16, tag="blob_b")
        nc.scalar.copy(out=blob_b, in_=blob_f)
        labt_b = cp.tile([BC, 16, 128], BF16, tag="labt_b")
        nc.scalar.copy(out=labt_b, in_=labt_f)
        slwt_b = cp.tile([BC, 16, D], BF16, tag="slwt_b")
        nc.scalar.copy(out=slwt_b, in_=slwt_f)

        # ================= med multi-hot (DVE) ==============================
        mb_a = cp.tile([BC, BC], BF16, tag="mb_a")
        nc.vector.tensor_scalar(out=mb_a, in0=med_fa, scalar1=0.9,
                                scalar2=None, op0=ALU.is_gt)
        mb_b = cp.tile([18, BC], BF16, tag="mb_b")
        nc.vector.tensor_scalar(out=mb_b, in0=med_fb, scalar1=0.9,
                                scalar2=None, op0=ALU.is_gt)

        # ================= glu encoder matmuls (PE block-diagonal) ==========
        gx_ps = psg.tile([BC, T, H], F32, tag="gx")
        for c in range(3):
            nc.tensor.matmul(gx_ps[:, 8 * c:8 * c + 8, :],
                             lhsT=glut_b[:, c, :],
                             rhs=blob_b[:, C_WBDG:C_WBDG + 256],
                             start=True, stop=False)
            nc.tensor.matmul(gx_ps[:, 8 * c:8 * c + 8, :],
                             lhsT=tft_b[:, c, :],
                             rhs=blob_b[:, C_WBDT:C_WBDT + 256],
                             start=False, stop=True)
        nc.tensor.matmul(gx_ps[:, 24, :], lhsT=glut_b[0:16, 3, :],
                         rhs=blob_b[0:16, C_GW3G:C_GW3G + 32],
                         start=True, stop=False)
        nc.tensor.matmul(gx_ps[:, 24, :], lhsT=tft_b[0:16, 3, :],
                         rhs=blob_b[0:16, C_GW3T:C_GW3T + 32],
                         start=False, stop=True)

        # med x0 = multihot @ med_w + med_b (bias row folded via ones row)
        x0_ps = ps.tile([BC, D], F32, tag="acc")
        nc.tensor.matmul(x0_ps, lhsT=mb_a,
                         rhs=blob_b[:, C_MWA:C_MWA + D], start=True, stop=False)
        nc.tensor.matmul(x0_ps, lhsT=mb_b,
                         rhs=blob_b[0:18, C_MWB:C_MWB + D], start=False, stop=True)

        # ================= weight prep (PE + gpsimd copies) =================
        # A_h[:, :32] = wq_h wk_h^T / 4 (glu columns only), stacked [64,(h,32)]
        a_ps = ps.tile([D, NH, H], F32, tag="acc")
        for h in range(NH):
            nc.tensor.matmul(a_ps[:, h, :],
                             lhsT=blob_b[0:16, C_WQT + 64 * h:C_WQT + 64 * h + 64],
                             rhs=blob_b[0:16, C_WKT + 32 * h:C_WKT + 32 * h + 32])
        # Wvo2 = m2_wv @ m2_wo
        wvo_ps = ps.tile([D, D], F32, tag="acc")
        nc.tensor.matmul(wvo_ps, lhsT=blob_b[0:D, C_M2WVT:C_M2WVT + D],
                         rhs=blob_b[0:D, C_M2WO:C_M2WO + D])

        a_sb = cp.tile([D, NH, H], BF16, tag="a_sb")
        nc.scalar.activation(out=a_sb, in_=a_ps, func=AF.Copy, scale=0.25)
        wvo_sb = cp.tile([D, D], BF16, tag="wvo_sb")
        nc.scalar.copy(out=wvo_sb, in_=wvo_ps)

        # t_h = wo_h @ Wvo2  [16,(h),64]
        t_ps = ps.tile([16, NH, D], F32, tag="acc")
        for h in range(NH):
            nc.tensor.matmul(t_ps[:, h, :],
                             lhsT=blob_b[0:D, C_WOT + 16 * h:C_WOT + 16 * h + 16],
                             rhs=wvo_sb)
        t_sb = cp.tile([16, NH, D], BF16, tag="t_sb")
        nc.scalar.copy(out=t_sb, in_=t_ps)
        # mwT[e, h, f] = MW_h[f, e] = (wv_h t_h)[f, e], computed transposed
        mwt_ps = ps.tile([D, NH, D], F32, tag="acc")
        for h in range(NH):
            nc.tensor.matmul(mwt_ps[:, h, :], lhsT=t_sb[:, h, :],
                             rhs=blob_b[0:16, C_WVT + 64 * h:C_WVT + 64 * h + 64])
        mwtg_sb = cp.tile([D, NH, H], BF16, tag="mwtg_sb")
        nc.scalar.copy(out=mwtg_sb, in_=mwt_ps[:, :, 0:H])
        mwth_sb = cp.tile([D, NH, H], BF16, tag="mwth_sb")
        nc.scalar.copy(out=mwth_sb, in_=mwt_ps[:, :, H:D])
        # MWg_stack[(h,f<32), e] via one PE transpose of mwT[:, :, :32]
        mwg_ps = ps.tile([128, D], BF16, tag="acc")
        nc.tensor.transpose(mwg_ps, mwtg_sb[:].rearrange("p h f -> p (h f)"),
                            ident[0:D, 0:D])
        mwg_sb = cp.tile([128, D], BF16, tag="mwg_sb")
        nc.scalar.copy(out=mwg_sb, in_=mwg_ps)
        # SMW = sum_h MW_h[32:, :]: sum mwT halves then transpose
        smwt = cp.tile([D, H], BF16, tag="smwt")
        nc.gpsimd.tensor_tensor(out=smwt, in0=mwth_sb[:, 0, :],
                                in1=mwth_sb[:, 1, :], op=ALU.add)
        nc.gpsimd.tensor_tensor(out=smwt, in0=smwt, in1=mwth_sb[:, 2, :],
                                op=ALU.add)
        nc.gpsimd.tensor_tensor(out=smwt, in0=smwt, in1=mwth_sb[:, 3, :],
                                op=ALU.add)
        smw_ps = ps.tile([H, D], BF16, tag="acc")
        nc.tensor.transpose(smw_ps, smwt[:], ident[0:D, 0:D])
        smw_sb = cp.tile([H, D], BF16, tag="smw_sb")
        nc.scalar.copy(out=smw_sb, in_=smw_ps)
        st1rt = cp.tile([D + 1, BC], BF16, tag="st1rt")
        nc.gpsimd.memset(st1rt[D:D + 1, :], 1.0)

        # ================= W1s partial reduce + AllReduce ===================
        # (emitted FIRST on the DVE queue after the multihot ops so the AR
        # input is ready ~34us on every core - the global AllReduce completes
        # when the slowest core contributes)
        def tree(lo, n):  # reduce w1m_b[:, lo:lo+n] into w1m_b[:, lo]
            while n > 1:
                half, odd = n // 2, n % 2
                if odd:
                    nc.vector.tensor_add(w1m_b[:, lo:lo + 1, :],
                                         w1m_b[:, lo:lo + 1, :],
                                         w1m_b[:, lo + n - 1:lo + n, :])
                    n -= 1
                nc.vector.tensor_add(w1m_b[:, lo:lo + half, :],
                                     w1m_b[:, lo:lo + half, :],
                                     w1m_b[:, lo + half:lo + 2 * half, :])
                n = half

        tree(0, 10)
        tree(10, 9)
        w1p = cp.tile([128, 580], BF16, tag="w1p")
        nc.vector.tensor_add(w1p[:].unsqueeze(1), w1m_b[:, 0:1, :],
                             w1m_b[:, 10:11, :])
        nc.sync.dma_start(
            out=cc_in[:].rearrange("d (r q) -> r d q", r=2), in_=w1p)
        nc.gpsimd.collective_compute(
            "AllReduce", ALU.add, replica_groups=[list(range(NC_CORES))],
            ins=[cc_in[:]], outs=[cc_out[:]])

        # ================= med gating -> mr0 -> u (critical chain) ==========
        scr = cp.tile([BC, D], BF16, tag="scr")
        nc.vector.tensor_mul(scr, x0_ps, blob_b[:, C_MG:C_MG + D])
        g0 = cp.tile([BC, 1], F32, tag="g0")
        nc.vector.tensor_reduce(out=g0, in_=scr, axis=AX.X, op=ALU.add)
        sg0 = cp.tile([BC, 1], F32, tag="sg0")
        nc.scalar.activation(out=sg0, in_=g0, func=AF.Sigmoid)
        mr0 = cp.tile([BC, D], BF16, tag="mr0")
        nc.vector.tensor_scalar(out=mr0, in0=x0_ps, scalar1=sg0[:, 0:1],
                                scalar2=None, op0=ALU.mult)
        mr0t_ps = ps.tile([D, BC], BF16, tag="acc")
        nc.tensor.transpose(mr0t_ps, mr0[:], ident[:])
        mr0t = cp.tile([D, BC], BF16, tag="mr0t")
        nc.vector.tensor_copy(out=mr0t, in_=mr0t_ps)
        u_ps = ps.tile([BC, NH, H], F32, tag="acc")
        nc.tensor.matmul(u_ps[:].rearrange("p h e -> p (h e)"), lhsT=mr0t,
                         rhs=a_sb[:].rearrange("p h e -> p (h e)"))
        u_sb = cp.tile([BC, NH, H], BF16, tag="u_sb")
        nc.vector.tensor_copy(out=u_sb, in_=u_ps)

        # ================= glu encoder tail (DVE/Act) =======================
        gxb = cp.tile([BC, T, H], BF16, tag="gxb")
        nc.vector.tensor_add(gxb, gx_ps,
                             blob_b[:, C_GB:C_GB + H].unsqueeze(1)
                             .broadcast_to((BC, T, H)))
        grep = cp.tile([BC, T, H], BF16, tag="grep")
        nc.scalar.activation(out=grep, in_=gxb, func=AF.Tanh)
        gm = cp.tile([BC, T, H], BF16, tag="gm")
        nc.vector.tensor_mul(gm, grep,
                             blob_b[:, C_GG:C_GG + H].unsqueeze(1)
                             .broadcast_to((BC, T, H)))
        gs = cp.tile([BC, T], F32, tag="gs")
        nc.vector.tensor_reduce(out=gs, in_=gm, axis=AX.X, op=ALU.add)
        gsg = cp.tile([BC, T], BF16, tag="gsg")
        nc.scalar.activation(out=gsg, in_=gs, func=AF.Sigmoid)
        nc.vector.tensor_mul(grep, grep,
                             gsg[:].unsqueeze(2).broadcast_to((BC, T, H)))

        # ================= one-query attention (glu dims only) ==============
        sprod = cp.tile([BC, NH, T, H], BF16, tag="sprod")
        nc.vector.tensor_mul(
            sprod,
            grep[:].unsqueeze(1).broadcast_to((BC, NH, T, H)),
            u_sb[:].unsqueeze(2).broadcast_to((BC, NH, T, H)))
        # halving-tree reduce over f (keeps fast bf16 DVE throughput)
        for wdt in (16, 8, 4, 2, 1):
            nc.vector.tensor_add(sprod[:, :, :, 0:wdt], sprod[:, :, :, 0:wdt],
                                 sprod[:, :, :, wdt:2 * wdt])
        es = cp.tile([BC, NH, T], BF16, tag="es")
        nc.scalar.activation(out=es, in_=sprod[:, :, :, 0], func=AF.Exp)
        den = cp.tile([BC, NH], F32, tag="den")
        nc.vector.tensor_reduce(out=den, in_=es, axis=AX.X, op=ALU.add)
        rden = cp.tile([BC, NH], F32, tag="rden")
        nc.vector.reciprocal(out=rden, in_=den)
        attn = cp.tile([BC, NH, T], BF16, tag="attn")
        nc.vector.tensor_mul(attn, es,
                             rden[:].unsqueeze(2).broadcast_to((BC, NH, T)))
        # weighted sum over visits, f-major so innermost stays packed
        grept = cp.tile([BC, H, T], BF16, tag="grept")
        nc.vector.tensor_copy(out=grept,
                              in_=grep[:].rearrange("p j f -> p f j"))
        wprod = cp.tile([BC, NH, H, T], BF16, tag="wprod")
        nc.vector.tensor_mul(
            wprod,
            attn[:].unsqueeze(2).broadcast_to((BC, NH, H, T)),
            grept[:].unsqueeze(1).broadcast_to((BC, NH, H, T)))
        nc.vector.tensor_add(wprod[:, :, :, 0:9], wprod[:, :, :, 0:9],
                             wprod[:, :, :, 16:25])
        for wdt in (8, 4, 2, 1):
            nc.vector.tensor_add(wprod[:, :, :, 0:wdt], wprod[:, :, :, 0:wdt],
                                 wprod[:, :, :, wdt:2 * wdt])
        wfin = cp.tile([BC, NH, H], BF16, tag="wfin")
        nc.vector.tensor_copy(out=wfin[:].unsqueeze(3), in_=wprod[:, :, :, 0:1])

        # ================= static MLP (transposed; no lab transpose) ========
        st1_ps = pst1.tile([D, BC], F32, tag="st1")
        for t in range(16):
            nc.tensor.matmul(st1_ps, lhsT=slwt_b[:, t, :], rhs=labt_b[:, t, :],
                             start=(t == 0), stop=(t == 15))
        nc.scalar.activation(out=st1rt[0:D, :], in_=st1_ps, func=AF.Relu)
        stat_ps = ps.tile([H, BC], F32, tag="acc")
        nc.tensor.matmul(stat_ps, lhsT=blob_b[0:D + 1, C_SLW2:C_SLW2 + H],
                         rhs=st1rt)
        statt = cp.tile([H, BC], BF16, tag="statt")
        nc.scalar.activation(out=statt, in_=stat_ps, func=AF.Relu)

        # ================= r = attention out + static part ==================
        wgt_ps = ps.tile([128, BC], BF16, tag="acc")
        nc.tensor.transpose(wgt_ps, wfin[:].rearrange("p h f -> p (h f)"),
                            ident[:])
        wgt_sb = cp.tile([128, BC], BF16, tag="wgt_sb")
        nc.vector.tensor_copy(out=wgt_sb, in_=wgt_ps)
        r_ps = ps.tile([BC, D], F32, tag="acc")
        nc.tensor.matmul(r_ps, lhsT=statt, rhs=smw_sb, start=True, stop=False)
        nc.tensor.matmul(r_ps, lhsT=wgt_sb, rhs=mwg_sb, start=False, stop=True)
        rr = cp.tile([BC, D], BF16, tag="rr")
        nc.scalar.activation(out=rr, in_=r_ps, func=AF.Relu)
        rrt_ps = ps.tile([D, BC], BF16, tag="acc")
        nc.tensor.transpose(rrt_ps, rr[:], ident[:])
        rrt = cp.tile([D, BC], BF16, tag="rrt")
        nc.vector.tensor_copy(out=rrt, in_=rrt_ps)

        # w2 cast late on Act queue (only needed after the AllGather)
        w2t_b = cp.tile([BC, 10, MED], BF16, tag="w2t_b")
        nc.scalar.copy(out=w2t_b, in_=w2t_f)

        # ================= final MLP (after AllGather) ======================
        # chunked bf16 readback: each chunk lands and feeds its hidT matmuls
        # while later chunks are still in flight (no cast needed)
        w1s_sb = cp.tile([D, HID], BF16, tag="w1s_sb")
        for o, n in ((0, 512), (512, 512), (1024, 136)):
            nc.sync.dma_start(out=w1s_sb[:, o:o + n], in_=cc_out[:, o:o + n])
        hidt = cp.tile([128, 10, 128], BF16, tag="hidt")
        nc.gpsimd.memset(hidt[:, 9, :], 1.0)  # row 8 stays 1.0 = b2 ones row
        # phase 1: all hidT matmuls + relus (pipelined, no out-matmul in between)
        for t in range(10):
            w = 128 if t < 9 else 8
            h_ps = ps.tile([128, BC], F32, tag="acc")
            nc.tensor.matmul(h_ps[0:w, :],
                             lhsT=w1s_sb[:, 128 * t:128 * t + w], rhs=rrt)
            nc.scalar.activation(out=hidt[0:w, t, :], in_=h_ps[0:w, :],
                                 func=AF.Relu,
                                 bias=blob_b[0:w, C_B1T + t:C_B1T + t + 1])
        # phase 2: back-to-back output accumulation (b2 rides w2t tile 9
        # row 8 against the ones row in hidt)
        out_ps = pout.tile([BC, MED], F32, tag="outacc")
        for t in range(10):
            w = 128 if t < 9 else 9
            nc.tensor.matmul(out_ps, lhsT=hidt[0:w, t, :],
                             rhs=w2t_b[0:w, t, :],
                             start=(t == 0), stop=(t == 9),
                             skip_group_check=True)
        out_sb = cp.tile([BC, MED], F32, tag="out_sb")
        nc.scalar.copy(out=out_sb, in_=out_ps)
        nc.scalar.dma_start(out=out_d[:], in_=out_sb)

    if split_waits:
        split_multi_waits(nc)
    return nc


_CACHED_NC = None


def make_in_maps(inputs):
    """Host-side input marshalling: pure layout work (transpose / pad / concat
    / replicate / constant fill) - every arithmetic op stays on device."""
    f = lambda x: np.ascontiguousarray(np.asarray(x, dtype=np.float32))
    cat = np.concatenate

    # out_w1 [9280, 1160] -> [145, 64, 1160] -> per-core m-shard of 19 blocks,
    # laid out [(half, d), m, 580] so the on-device tree reduce is contiguous
    w1 = f(inputs["out_w1"]).reshape(MED, D, HID)
    w1pad = np.zeros((8 * 19, D, HID), np.float32)
    w1pad[:MED] = w1

    # lab^T k-tiles with ones column folded at row 1957
    lab = f(inputs["lab"])
    lab_ext = np.zeros((B, 2048), np.float32)
    lab_ext[:, :LAB] = lab
    lab_ext[:, LAB] = 1.0           # ones column folds sll_b1 into the matmul

    slw = np.zeros((2048, D), np.float32)
    slw[:LAB] = f(inputs["sll_w1"])
    slw[LAB] = f(inputs["sll_b1"])
    slwt = np.ascontiguousarray(slw.reshape(16, 128, D).transpose(1, 0, 2))

    glu, tf = f(inputs["glu"]), f(inputs["time_feat"])

    def jf_major(x):  # (j, f)-major transpose, padded 400 -> 512
        z = np.zeros((B, 512), np.float32)
        z[:, :T * GLU] = x.reshape(B, T * GLU)
        return z.reshape(B, 4, 128).transpose(2, 1, 0)  # [128p, 4c, B]

    glut, tft = jf_major(glu), jf_major(tf)

    med0 = f(inputs["med"])[:, 0, :]                  # [B, 145]
    medt = np.ones((MED + 1, B), np.float32)          # row 145 = 1.0
    medt[:MED] = med0.T

    w2 = np.zeros((1280, MED), np.float32)
    w2[:HID] = f(inputs["out_w2"])
    w2[HID] = f(inputs["out_b2"])          # rides k-tile 9 row 8 (ones in hidt)
    w2t = np.ascontiguousarray(w2.reshape(10, 128, MED).transpose(1, 0, 2))

    # ---- packed weight blob ----
    blob = np.zeros((BC, BK), np.float32)
    glu_w = f(inputs["glu_w"])                         # [32, 32]
    for jl in range(8):
        blob[16 * jl:16 * jl + 16,
             C_WBDG + 32 * jl:C_WBDG + 32 * jl + 32] = glu_w[:16]
        blob[16 * jl:16 * jl + 16,
             C_WBDT + 32 * jl:C_WBDT + 32 * jl + 32] = glu_w[16:]
    blob[:, C_GG:C_GG + H] = f(inputs["glu_gate"])[None, :]
    blob[:, C_GB:C_GB + H] = f(inputs["glu_b"])[None, :]
    blob[:, C_MG:C_MG + D] = f(inputs["med_gate"])[None, :]
    medw_ext = cat([f(inputs["med_w"]), f(inputs["med_b"])[None, :]], 0)
    blob[:, C_MWA:C_MWA + D] = medw_ext[:128]
    blob[0:18, C_MWB:C_MWB + D] = medw_ext[128:]
    b1 = f(inputs["out_b1"])
    for t in range(10):
        nvalid = 128 if t < 9 else 8
        blob[:nvalid, C_B1T + t] = b1[128 * t:128 * t + nvalid]
    blob[0, C_B2:C_B2 + MED] = f(inputs["out_b2"])
    wo, wv = f(inputs["m1_wo"]), f(inputs["m1_wv"])
    wq, wk = f(inputs["m1_wq"]), f(inputs["m1_wk"])
    blob[0:D, C_WOT:C_WOT + D] = wo.T                  # woT[d, (h,g)]
    blob[0:D, C_M2WVT:C_M2WVT + D] = f(inputs["m2_wv"]).T
    blob[0:D, C_M2WO:C_M2WO + D] = f(inputs["m2_wo"])
    blob[0:D, C_SLW2:C_SLW2 + H] = f(inputs["sll_w2"])
    blob[D, C_SLW2:C_SLW2 + H] = f(inputs["sll_b2"])
    for h in range(NH):
        blob[0:16, C_WQT + 64 * h:C_WQT + 64 * h + 64] = wq[:, 16 * h:16 * h + 16].T
        blob[0:16, C_WKT + 32 * h:C_WKT + 32 * h + 32] = wk[:H, 16 * h:16 * h + 16].T
        blob[0:16, C_WVT + 64 * h:C_WVT + 64 * h + 64] = wv[:, 16 * h:16 * h + 16].T
    blob[0:16, C_GW3G:C_GW3G + H] = glu_w[:16]
    blob[0:16, C_GW3T:C_GW3T + H] = glu_w[16:]

    in_maps = []
    for c in range(NC_CORES):
        sl = slice(c * BC, (c + 1) * BC)
        in_maps.append({
            "w1m": np.ascontiguousarray(
                w1pad[19 * c:19 * c + 19].reshape(19, D, 2, 580)
                .transpose(2, 1, 0, 3).reshape(128, 19, 580)),
            "labt": np.ascontiguousarray(
                lab_ext[sl].T.reshape(16, 128, BC).transpose(1, 0, 2)),
            "slwt": slwt,
            "glut": np.ascontiguousarray(glut[:, :, sl]),
            "tft": np.ascontiguousarray(tft[:, :, sl]),
            "medt": np.ascontiguousarray(medt[:, sl]),
            "w2t": w2t,
            "blob": blob,
        })
    return in_maps


def kernel(**inputs):
    global _CACHED_NC
    if _CACHED_NC is None:
        _CACHED_NC = build_bass()
    nc = _CACHED_NC
    in_maps = make_in_maps(inputs)
    res = run_bass_kernel_spmd(nc, in_maps, core_ids=list(range(NC_CORES)))
    return np.concatenate([res.results[c]["out"] for c in range(NC_CORES)], axis=0)


if __name__ == "__main__":
    import reference
    inp = reference.setup_inputs()
    out = kernel(**{k: np.asarray(v) for k, v in inp.items()})
    print("kernel output", out.shape, out.dtype)
